# revision 1
# baseline (speedup 1.0000x reference)
"""2-layer GCN + classifier on 8 Trainium2 NeuronCores.

Strategy (graph/data parallel per sharding hint):
- Nodes sharded 8 ways by contiguous range (12500/core). Edges partitioned by
  dst shard on host, grouped by (dst-tile of 128, src-chunk of 32768), padded
  to 128-edge blocks. Self-loops appended as ordinary edges.
- Per GCN layer: each core builds its shard of the gather table
  T = dinv * (Z @ W) (bf16), AllGather -> full table in every core's HBM.
  Aggregation: dma_gather rows by src (int16 chunk-local idx), build one-hot
  dst-slot matrices via iota+is_equal, segment-sum via PSUM-accumulated bf16
  matmuls. Epilogue: dinv[dst]*agg + b, ELU, next W matmul (hi/lo split bf16
  pairs recover ~f32 weight precision), dinv pre-scale for next table.
- Classifier + log_softmax computed per node tile on device; host concatenates
  the 8 output shards.
"""
import sys

sys.path.insert(0, "/opt/trn_rl_repo")

import numpy as np
import ml_dtypes

import concourse.bacc as bacc
import concourse.tile as tile
from concourse import mybir
from concourse.bass_utils import run_bass_kernel_spmd

# ---------------- problem constants (hardcoded per task statement) ----------
N = 100000
E = 1600000
F_IN = 128
HID = 128
C_OUT = 40
NCORES = 8
NSH = N // NCORES          # 12500 nodes per core
P = 128
NT = (NSH + P - 1) // P    # 98 dst tiles per core (last has 84 rows)
NSH_PAD = NT * P           # 12544
CH = 32768                 # gather chunk rows (int16 idx limit)
NCHUNK = (N + CH - 1) // CH  # 4
TG = 12                    # dst tiles per tile-group (PSUM: 3 banks x 2 bufs)
NTG = (NT + TG - 1) // TG  # 9 tile groups

F32 = mybir.dt.float32
BF16 = mybir.dt.bfloat16
I16 = mybir.dt.int16


def _split_hi_lo(w):
    hi = w.astype(ml_dtypes.bfloat16)
    lo = (w - hi.astype(np.float32)).astype(ml_dtypes.bfloat16)
    return hi, lo


def _prep_host(x, edge_index, W0, b0, W1, b1, Wl, bl):
    """Shard + reorder edges; build all per-core device input arrays."""
    src = np.asarray(edge_index[0]).astype(np.int64)
    dst = np.asarray(edge_index[1]).astype(np.int64)
    loop = np.arange(N, dtype=np.int64)
    src2 = np.concatenate([src, loop])
    dst2 = np.concatenate([dst, loop])
    deg = np.bincount(dst2, minlength=N).astype(np.float32)  # = indeg + 1

    # ---- per-core edge grouping by (dst_tile, src_chunk) ----
    counts = np.zeros((NCORES, NT, NCHUNK), dtype=np.int64)
    grouped = []  # per core: (sorted_src_local int64, sorted_dstslot int64)
    core_of = dst2 // NSH
    for c in range(NCORES):
        sel = core_of == c
        es = src2[sel]
        ed = dst2[sel] - c * NSH
        tile_id = ed // P
        chunk_id = es // CH
        key = tile_id * NCHUNK + chunk_id
        order = np.argsort(key, kind="stable")
        es, ed, key = es[order], ed[order], key[order]
        cnt = np.bincount(key, minlength=NT * NCHUNK).reshape(NT, NCHUNK)
        counts[c] = cnt
        grouped.append((es, ed % P, cnt))

    bmax = np.ceil(counts.max(axis=0) / P).astype(np.int64)  # [NT, NCHUNK] blocks
    nblk_tk = bmax  # blocks per (t, k), shared across cores

    # flatten order: for tg: for k: for t in tg: blocks
    # Build per-core gidx (chunk-local src, wrapped) and dstslot streams.
    tot_blocks = 0
    for g in range(NTG):
        tiles = range(g * TG, min((g + 1) * TG, NT))
        for k in range(NCHUNK):
            for t in tiles:
                tot_blocks += int(nblk_tk[t][k])
    tot_slots = tot_blocks * P

    gidx_all = np.zeros((NCORES, 128, tot_slots // 16), dtype=np.int16)
    dsl_all = np.full((NCORES, 128, tot_blocks), -1.0, dtype=np.float32)

    for c in range(NCORES):
        es, slots, cnt = grouped[c]
        starts = np.zeros(NT * NCHUNK + 1, dtype=np.int64)
        np.cumsum(cnt.reshape(-1), out=starts[1:])
        gcol = 0  # gidx column offset (16-wide units)
        bcol = 0  # dstslot block column offset
        for g in range(NTG):
            tiles = range(g * TG, min((g + 1) * TG, NT))
            for k in range(NCHUNK):
                for t in tiles:
                    nb = int(nblk_tk[t][k])
                    if nb == 0:
                        continue
                    a, b = starts[t * NCHUNK + k], starts[t * NCHUNK + k + 1]
                    n = b - a
                    pad = nb * P - n
                    sl = np.concatenate(
                        [es[a:b] - k * CH, np.zeros(pad, dtype=np.int64)]
                    ).astype(np.int16)
                    ds = np.concatenate(
                        [slots[a:b], np.full(pad, -1, dtype=np.int64)]
                    ).astype(np.float32)
                    # wrap idx: slot i -> [i%16, i//16]
                    gidx_all[c, :16, gcol : gcol + nb * 8] = sl.reshape(-1, 16).T
                    dsl_all[c, :, bcol : bcol + nb] = ds.reshape(nb, P).T
                    gcol += nb * 8
                    bcol += nb
        gidx_all[c] = np.tile(gidx_all[c, :16], (8, 1))

    # degree layouts
    deg_col = np.ones((NCORES, 128, NT), dtype=np.float32)
    deg_row = np.ones((NCORES, 1, NSH_PAD), dtype=np.float32)
    for c in range(NCORES):
        d = deg[c * NSH : (c + 1) * NSH]
        dp = np.concatenate([d, np.ones(NSH_PAD - NSH, dtype=np.float32)])
        deg_col[c] = dp.reshape(NT, P).T
        deg_row[c, 0] = dp

    # x transposed shards, hi/lo bf16
    xT_hi = np.zeros((NCORES, 128, NSH_PAD), dtype=ml_dtypes.bfloat16)
    xT_lo = np.zeros((NCORES, 128, NSH_PAD), dtype=ml_dtypes.bfloat16)
    for c in range(NCORES):
        xs = np.asarray(x[c * NSH : (c + 1) * NSH]).astype(np.float32).T  # [128, NSH]
        hi, lo = _split_hi_lo(xs)
        xT_hi[c, :, :NSH] = hi
        xT_lo[c, :, :NSH] = lo

    W0h, W0l = _split_hi_lo(np.asarray(W0, dtype=np.float32))
    W1h, W1l = _split_hi_lo(np.asarray(W1, dtype=np.float32))
    Wlh, Wll = _split_hi_lo(np.asarray(Wl, dtype=np.float32))
    b0c = np.asarray(b0, dtype=np.float32).reshape(128, 1)
    b1c = np.asarray(b1, dtype=np.float32).reshape(128, 1)
    blb = np.tile(np.asarray(bl, dtype=np.float32).reshape(1, C_OUT), (128, 1))

    in_maps = []
    for c in range(NCORES):
        in_maps.append(
            {
                "xT_hi": xT_hi[c],
                "xT_lo": xT_lo[c],
                "gidx": gidx_all[c],
                "dsl": dsl_all[c],
                "deg_col": deg_col[c],
                "deg_row": deg_row[c],
                "W0h": W0h, "W0l": W0l,
                "W1h": W1h, "W1l": W1l,
                "Wlh": Wlh, "Wll": Wll,
                "b0c": b0c, "b1c": b1c, "blb": blb,
            }
        )
    return in_maps, nblk_tk, tot_blocks, tot_slots


def _build_program(nblk_tk, tot_blocks, tot_slots):
    nc = bacc.Bacc(num_devices=NCORES)
    xT_hi = nc.declare_dram_parameter("xT_hi", [128, NSH_PAD], BF16, isOutput=False)
    xT_lo = nc.declare_dram_parameter("xT_lo", [128, NSH_PAD], BF16, isOutput=False)
    gidx = nc.declare_dram_parameter("gidx", [128, tot_slots // 16], I16, isOutput=False)
    dsl = nc.declare_dram_parameter("dsl", [128, tot_blocks], F32, isOutput=False)
    deg_col = nc.declare_dram_parameter("deg_col", [128, NT], F32, isOutput=False)
    deg_row = nc.declare_dram_parameter("deg_row", [1, NSH_PAD], F32, isOutput=False)
    W0h = nc.declare_dram_parameter("W0h", [128, HID], BF16, isOutput=False)
    W0l = nc.declare_dram_parameter("W0l", [128, HID], BF16, isOutput=False)
    W1h = nc.declare_dram_parameter("W1h", [128, HID], BF16, isOutput=False)
    W1l = nc.declare_dram_parameter("W1l", [128, HID], BF16, isOutput=False)
    Wlh = nc.declare_dram_parameter("Wlh", [128, C_OUT], BF16, isOutput=False)
    Wll = nc.declare_dram_parameter("Wll", [128, C_OUT], BF16, isOutput=False)
    b0c = nc.declare_dram_parameter("b0c", [128, 1], F32, isOutput=False)
    b1c = nc.declare_dram_parameter("b1c", [128, 1], F32, isOutput=False)
    blb = nc.declare_dram_parameter("blb", [128, C_OUT], F32, isOutput=False)
    out_ext = nc.declare_dram_parameter("out", [NSH, C_OUT], F32, isOutput=True)

    t1_shard = nc.dram_tensor("t1_shard", [NSH, HID], BF16)
    t2_shard = nc.dram_tensor("t2_shard", [NSH, HID], BF16)
    T1_full = nc.dram_tensor("T1_full", [N, HID], BF16, addr_space="Shared")
    T2_full = nc.dram_tensor("T2_full", [N, HID], BF16, addr_space="Shared")

    # per-(tg,k) slot counts and offsets
    sgk = np.zeros((NTG, NCHUNK), dtype=np.int64)
    for g in range(NTG):
        tiles = range(g * TG, min((g + 1) * TG, NT))
        for k in range(NCHUNK):
            sgk[g][k] = P * sum(int(nblk_tk[t][k]) for t in tiles)
    max_gk_blocks = int(sgk.max()) // P

    from contextlib import ExitStack
    with tile.TileContext(nc) as tc, ExitStack() as es:
        cpool = es.enter_context(tc.tile_pool(name="const", bufs=1))
        xpool = es.enter_context(tc.tile_pool(name="xp", bufs=3))
        gpool = es.enter_context(tc.tile_pool(name="gp", bufs=2))
        ipool = es.enter_context(tc.tile_pool(name="ip", bufs=2))
        dpool = es.enter_context(tc.tile_pool(name="dp", bufs=2))
        spool = es.enter_context(tc.tile_pool(name="sp", bufs=6))
        zpool = es.enter_context(tc.tile_pool(name="zp", bufs=2))
        opool = es.enter_context(tc.tile_pool(name="op", bufs=2))
        apsum = es.enter_context(tc.tile_pool(name="apsum", bufs=2, space="PSUM"))
        wpsum = es.enter_context(tc.tile_pool(name="wpsum", bufs=2, space="PSUM"))

        # ---- constants ----
        iota_t = cpool.tile([P, P], BF16, tag="iota")
        nc.gpsimd.iota(iota_t[:], pattern=[[1, P]], base=0, channel_multiplier=0,
                       allow_small_or_imprecise_dtypes=True)
        w0h_t = cpool.tile([128, HID], BF16, tag="w0h")
        w0l_t = cpool.tile([128, HID], BF16, tag="w0l")
        w1h_t = cpool.tile([128, HID], BF16, tag="w1h")
        w1l_t = cpool.tile([128, HID], BF16, tag="w1l")
        wlh_t = cpool.tile([128, C_OUT], BF16, tag="wlh")
        wll_t = cpool.tile([128, C_OUT], BF16, tag="wll")
        b0_t = cpool.tile([128, 1], F32, tag="b0")
        b1_t = cpool.tile([128, 1], F32, tag="b1")
        blb_t = cpool.tile([128, C_OUT], F32, tag="blb")
        for tt, ext in [(w0h_t, W0h), (w0l_t, W0l), (w1h_t, W1h), (w1l_t, W1l),
                        (wlh_t, Wlh), (wll_t, Wll), (b0_t, b0c), (b1_t, b1c),
                        (blb_t, blb)]:
            nc.sync.dma_start(out=tt[:], in_=ext[:, :])

        # ---- dinv (column and broadcast layouts) ----
        dcol_raw = cpool.tile([128, NT], F32, tag="dcolr")
        nc.sync.dma_start(out=dcol_raw[:], in_=deg_col[:, :])
        dcol_s = cpool.tile([128, NT], F32, tag="dcols")
        nc.scalar.activation(dcol_s[:], dcol_raw[:], mybir.ActivationFunctionType.Sqrt)
        dinv_col = cpool.tile([128, NT], F32, tag="dcol")
        nc.vector.reciprocal(dinv_col[:], dcol_s[:])

        # dinv broadcast tile: row0=deg -> row1=sqrt -> row0=1/sqrt -> double up
        dinv_b = cpool.tile([128, NSH_PAD], F32, tag="dinvb")
        nc.sync.dma_start(out=dinv_b[:1, :], in_=deg_row[:, :])
        nc.scalar.activation(dinv_b[32:33, :], dinv_b[:1, :],
                             mybir.ActivationFunctionType.Sqrt)
        nc.vector.reciprocal(dinv_b[:1, :], dinv_b[32:33, :])
        rows_done = 1
        while rows_done < 128:
            n = min(rows_done, 128 - rows_done)
            nc.sync.dma_start(out=dinv_b[rows_done : rows_done + n, :],
                              in_=dinv_b[:n, :])
            rows_done += n

        # ---- phase 1: T1 shard = dinv * (X @ W0) ----
        for t in range(NT):
            rows = min(P, NSH - t * P)
            xh = xpool.tile([128, P], BF16, tag="xh")
            xl = xpool.tile([128, P], BF16, tag="xl")
            nc.sync.dma_start(out=xh[:], in_=xT_hi[:, t * P : (t + 1) * P])
            nc.sync.dma_start(out=xl[:], in_=xT_lo[:, t * P : (t + 1) * P])
            ps = wpsum.tile([P, HID], F32, tag="wps", space="PSUM")
            nc.tensor.matmul(out=ps[:], lhsT=xh[:], rhs=w0h_t[:], start=True, stop=False)
            nc.tensor.matmul(out=ps[:], lhsT=xh[:], rhs=w0l_t[:], start=False, stop=False)
            nc.tensor.matmul(out=ps[:], lhsT=xl[:], rhs=w0h_t[:], start=False, stop=True)
            tb = opool.tile([P, HID], BF16, tag="tb")
            nc.vector.tensor_scalar(out=tb[:], in0=ps[:], scalar1=dinv_col[:, t : t + 1],
                                    scalar2=None, op0=mybir.AluOpType.mult)
            nc.sync.dma_start(out=t1_shard[t * P : t * P + rows, :], in_=tb[:rows, :])

        # ---- allgather T1 ----
        nc.gpsimd.collective_compute(
            "AllGather", mybir.AluOpType.bypass,
            replica_groups=[list(range(NCORES))],
            ins=[t1_shard[:].opt()], outs=[T1_full[:].opt()],
        )

        # ---- aggregation layers ----
        def agg_layer(T_full, layer):
            gcol = [0]  # running gidx column offset (16-wide)
            bcol = [0]  # running dstslot block column
            for g in range(NTG):
                tiles = list(range(g * TG, min((g + 1) * TG, NT)))
                # per-(t) first/last chunk with blocks, for start/stop flags
                first_k = {}
                last_k = {}
                for t in tiles:
                    ks = [k for k in range(NCHUNK) if nblk_tk[t][k] > 0]
                    first_k[t], last_k[t] = ks[0], ks[-1]
                # allocate agg psum: bank tiles of [128, 512]; tile t -> slot
                nbank = (len(tiles) + 3) // 4
                banks = [apsum.tile([P, 512], F32, tag=f"agg{i}", space="PSUM",
                                    name=f"aggbank{i}")
                         for i in range(nbank)]

                def agg_ap(ti):
                    i = tiles.index(ti)
                    return banks[i // 4][:, (i % 4) * P : (i % 4 + 1) * P]

                # dstslot slab for this tile group
                nb_tg = sum(int(nblk_tk[t][k]) for t in tiles for k in range(NCHUNK))
                dst_t = dpool.tile([128, nb_tg], F32, tag="dsl")
                nc.sync.dma_start(out=dst_t[:], in_=dsl[:, bcol[0] : bcol[0] + nb_tg])

                dcol_off = 0
                for k in range(NCHUNK):
                    s_gk = int(sgk[g][k])
                    if s_gk == 0:
                        continue
                    kend = min((k + 1) * CH, N)
                    idxt = ipool.tile([128, max(int(sgk.max()) // 16, 16)], I16, tag="gidx")
                    nc.sync.dma_start(out=idxt[:, : s_gk // 16],
                                      in_=gidx[:, gcol[0] : gcol[0] + s_gk // 16])
                    gbuf = gpool.tile([P, max_gk_blocks, P], BF16, tag="gath")
                    nblk_gk = s_gk // P
                    nc.gpsimd.dma_gather(
                        gbuf[:, :nblk_gk, :], T_full[k * CH : kend, :],
                        idxt[:, : s_gk // 16], s_gk, s_gk, HID,
                        single_packet=False,
                    )
                    gcol[0] += s_gk // 16
                    boff = 0
                    for t in tiles:
                        nb = int(nblk_tk[t][k])
                        for b in range(nb):
                            s_t = spool.tile([P, P], BF16, tag="onehot")
                            nc.vector.tensor_scalar(
                                out=s_t[:], in0=iota_t[:],
                                scalar1=dst_t[:, dcol_off : dcol_off + 1],
                                scalar2=None, op0=mybir.AluOpType.is_equal,
                            )
                            nc.tensor.matmul(
                                out=agg_ap(t), lhsT=gbuf[:, boff, :], rhs=s_t[:],
                                start=(k == first_k[t] and b == 0),
                                stop=(k == last_k[t] and b == nb - 1),
                                skip_group_check=True,
                            )
                            boff += 1
                            dcol_off += 1
                bcol[0] += nb_tg

                # ---- epilogue per tile ----
                for t in tiles:
                    rows = min(P, NSH - t * P)
                    bias = b0_t if layer == 1 else b1_t
                    u = zpool.tile([P, P], F32, tag="u")
                    nc.vector.tensor_tensor(
                        out=u[:], in0=agg_ap(t),
                        in1=dinv_b[:, t * P : (t + 1) * P],
                        op=mybir.AluOpType.mult)
                    m = zpool.tile([P, P], F32, tag="m")
                    nc.vector.tensor_scalar(out=m[:], in0=u[:], scalar1=bias[:],
                                            scalar2=0.0, op0=mybir.AluOpType.add,
                                            op1=mybir.AluOpType.min)
                    pmax = zpool.tile([P, P], F32, tag="pmax")
                    nc.vector.tensor_scalar(out=pmax[:], in0=u[:], scalar1=bias[:],
                                            scalar2=0.0, op0=mybir.AluOpType.add,
                                            op1=mybir.AluOpType.max)
                    e = zpool.tile([P, P], F32, tag="e")
                    nc.scalar.activation(e[:], m[:], mybir.ActivationFunctionType.Exp)
                    zs = zpool.tile([P, P], F32, tag="zs")
                    nc.vector.tensor_tensor(out=zs[:], in0=pmax[:], in1=e[:],
                                            op=mybir.AluOpType.add)
                    zb = zpool.tile([P, P], BF16, tag="zb")
                    nc.vector.tensor_scalar(out=zb[:], in0=zs[:], scalar1=-1.0,
                                            scalar2=None, op0=mybir.AluOpType.add)
                    if layer == 1:
                        ps2 = wpsum.tile([P, HID], F32, tag="wps", space="PSUM")
                        nc.tensor.matmul(out=ps2[:], lhsT=zb[:], rhs=w1h_t[:],
                                         start=True, stop=False)
                        nc.tensor.matmul(out=ps2[:], lhsT=zb[:], rhs=w1l_t[:],
                                         start=False, stop=True)
                        t2b = opool.tile([P, HID], BF16, tag="tb")
                        nc.vector.tensor_scalar(out=t2b[:], in0=ps2[:],
                                                scalar1=dinv_col[:, t : t + 1],
                                                scalar2=None, op0=mybir.AluOpType.mult)
                        nc.sync.dma_start(out=t2_shard[t * P : t * P + rows, :],
                                          in_=t2b[:rows, :])
                    else:
                        ps3 = wpsum.tile([P, HID], F32, tag="wps", space="PSUM")
                        nc.tensor.matmul(out=ps3[:, :C_OUT], lhsT=zb[:], rhs=wlh_t[:],
                                         start=True, stop=False)
                        nc.tensor.matmul(out=ps3[:, :C_OUT], lhsT=zb[:], rhs=wll_t[:],
                                         start=False, stop=True)
                        lg = opool.tile([P, C_OUT], F32, tag="lg")
                        nc.vector.tensor_tensor(out=lg[:], in0=ps3[:, :C_OUT],
                                                in1=blb_t[:], op=mybir.AluOpType.add)
                        mx = opool.tile([P, 1], F32, tag="mx")
                        nc.vector.tensor_reduce(out=mx[:], in_=lg[:],
                                                axis=mybir.AxisListType.X,
                                                op=mybir.AluOpType.max)
                        sh = opool.tile([P, C_OUT], F32, tag="sh")
                        nc.vector.tensor_scalar(out=sh[:], in0=lg[:], scalar1=mx[:],
                                                scalar2=None,
                                                op0=mybir.AluOpType.subtract)
                        ex = opool.tile([P, C_OUT], F32, tag="ex")
                        sm = opool.tile([P, 1], F32, tag="sm")
                        nc.scalar.activation(ex[:], sh[:],
                                             mybir.ActivationFunctionType.Exp,
                                             accum_out=sm[:])
                        ln = opool.tile([P, 1], F32, tag="ln")
                        nc.scalar.activation(ln[:], sm[:],
                                             mybir.ActivationFunctionType.Ln)
                        res = opool.tile([P, C_OUT], F32, tag="res")
                        nc.vector.tensor_scalar(out=res[:], in0=sh[:], scalar1=ln[:],
                                                scalar2=None,
                                                op0=mybir.AluOpType.subtract)
                        nc.sync.dma_start(out=out_ext[t * P : t * P + rows, :],
                                          in_=res[:rows, :])

        agg_layer(T1_full, 1)
        nc.gpsimd.collective_compute(
            "AllGather", mybir.AluOpType.bypass,
            replica_groups=[list(range(NCORES))],
            ins=[t2_shard[:].opt()], outs=[T2_full[:].opt()],
        )
        agg_layer(T2_full, 2)

    nc.finalize()
    return nc


_CACHE = {}


def kernel(**inputs):
    in_maps, nblk_tk, tot_blocks, tot_slots = _prep_host(
        inputs["x"], inputs["edge_index"], inputs["W0"], inputs["b0"],
        inputs["W1"], inputs["b1"], inputs["Wl"], inputs["bl"])
    key = (tot_blocks, tot_slots, nblk_tk.tobytes())
    if key not in _CACHE:
        _CACHE[key] = _build_program(nblk_tk, tot_blocks, tot_slots)
    nc = _CACHE[key]
    trace = bool(int(__import__("os").environ.get("KERNEL_TRACE", "0")))
    res = run_bass_kernel_spmd(nc, in_maps, list(range(NCORES)), trace=trace)
    kernel.last_results = res
    out = np.concatenate([res.results[c]["out"] for c in range(NCORES)], axis=0)
    return out.astype(np.float32)



# revision 2
# speedup vs baseline: 1.1988x; 1.1988x over previous
"""2-layer GCN + classifier on 8 Trainium2 NeuronCores.

Strategy (graph/data parallel):
- Nodes sharded 8 ways by contiguous range (12500/core). Edges partitioned by
  dst shard on host, grouped by (dst-tile of 128, src-chunk of 32768), padded
  to 128-edge blocks. Self-loops appended as ordinary edges.
- Per GCN layer: each core builds its shard of the gather table
  T = dinv * (Z @ W) (bf16), AllGather -> full table in every core's HBM.
  Aggregation: dma_gather rows by src (int16 chunk-local idx) spread over the
  4 SWDGE queues (overlaps descriptor generation with drains), one-hot
  dst-slot matrices built in wide batches via broadcast-AP is_equal,
  segment-sum via PSUM-accumulated bf16 matmuls. Bias (pre-scaled by
  sqrt(deg)) seeds the PSUM banks via a rank-1 matmul so the epilogue is just
  dinv-scale + ELU, batched per 12-tile group.
- Classifier + log_softmax computed per tile group; host concatenates the 8
  output shards.
"""
import sys

sys.path.insert(0, "/opt/trn_rl_repo")

import numpy as np
import ml_dtypes

import concourse.bacc as bacc
import concourse.tile as tile
from concourse import mybir
from concourse.bass_utils import run_bass_kernel_spmd

# ---------------- problem constants (hardcoded per task statement) ----------
N = 100000
E = 1600000
F_IN = 128
HID = 128
C_OUT = 40
NCORES = 8
NSH = N // NCORES          # 12500 nodes per core
P = 128
NT = (NSH + P - 1) // P    # 98 dst tiles per core (last has 84 rows)
NSH_PAD = NT * P           # 12544
CH = 32768                 # gather chunk rows (int16 idx limit)
NCHUNK = (N + CH - 1) // CH  # 4
TG = 12                    # dst tiles per tile-group (PSUM: 3 banks x 2 bufs)
NTG = (NT + TG - 1) // TG  # 9 tile groups
OB = 24                    # one-hot build batch (blocks per DVE op)
NQ = 4                     # SWDGE queues for gathers

F32 = mybir.dt.float32
BF16 = mybir.dt.bfloat16
I16 = mybir.dt.int16


def _split_hi_lo(w):
    hi = w.astype(ml_dtypes.bfloat16)
    lo = (w - hi.astype(np.float32)).astype(ml_dtypes.bfloat16)
    return hi, lo


def _prep_host(x, edge_index, W0, b0, W1, b1, Wl, bl):
    """Shard + reorder edges; build all per-core device input arrays."""
    src = np.asarray(edge_index[0]).astype(np.int64)
    dst = np.asarray(edge_index[1]).astype(np.int64)
    loop = np.arange(N, dtype=np.int64)
    src2 = np.concatenate([src, loop])
    dst2 = np.concatenate([dst, loop])
    deg = np.bincount(dst2, minlength=N).astype(np.float32)  # = indeg + 1

    # ---- per-core edge grouping by (dst_tile, src_chunk) ----
    counts = np.zeros((NCORES, NT, NCHUNK), dtype=np.int64)
    grouped = []  # per core: (sorted_src, dst_slot, cnt)
    core_of = dst2 // NSH
    for c in range(NCORES):
        sel = core_of == c
        es = src2[sel]
        ed = dst2[sel] - c * NSH
        tile_id = ed // P
        chunk_id = es // CH
        key = tile_id * NCHUNK + chunk_id
        order = np.argsort(key, kind="stable")
        es, ed, key = es[order], ed[order], key[order]
        cnt = np.bincount(key, minlength=NT * NCHUNK).reshape(NT, NCHUNK)
        counts[c] = cnt
        grouped.append((es, ed % P, cnt))

    nblk_tk = np.ceil(counts.max(axis=0) / P).astype(np.int64)  # [NT, NCHUNK]

    # flatten order: for tg: for k: for t in tg: blocks
    tot_blocks = 0
    for g in range(NTG):
        tiles = range(g * TG, min((g + 1) * TG, NT))
        for k in range(NCHUNK):
            for t in tiles:
                tot_blocks += int(nblk_tk[t][k])
    tot_slots = tot_blocks * P

    gidx_all = np.zeros((NCORES, 128, tot_slots // 16), dtype=np.int16)
    dsl_all = np.full((NCORES, 128, tot_blocks), -1.0, dtype=ml_dtypes.bfloat16)

    for c in range(NCORES):
        es, slots, cnt = grouped[c]
        starts = np.zeros(NT * NCHUNK + 1, dtype=np.int64)
        np.cumsum(cnt.reshape(-1), out=starts[1:])
        gcol = 0  # gidx column offset (16-wide units)
        bcol = 0  # dstslot block column offset
        for g in range(NTG):
            tiles = range(g * TG, min((g + 1) * TG, NT))
            for k in range(NCHUNK):
                for t in tiles:
                    nb = int(nblk_tk[t][k])
                    if nb == 0:
                        continue
                    a, b = starts[t * NCHUNK + k], starts[t * NCHUNK + k + 1]
                    n = b - a
                    pad = nb * P - n
                    sl = np.concatenate(
                        [es[a:b] - k * CH, np.zeros(pad, dtype=np.int64)]
                    ).astype(np.int16)
                    ds = np.concatenate(
                        [slots[a:b], np.full(pad, -1, dtype=np.int64)]
                    ).astype(ml_dtypes.bfloat16)
                    # wrap idx: slot i -> [i%16, i//16]
                    gidx_all[c, :16, gcol : gcol + nb * 8] = sl.reshape(-1, 16).T
                    dsl_all[c, :, bcol : bcol + nb] = ds.reshape(nb, P).T
                    gcol += nb * 8
                    bcol += nb
        gidx_all[c] = np.tile(gidx_all[c, :16], (8, 1))

    # degree-derived tables (host-precomputed)
    dinv = np.where(deg > 0, 1.0 / np.sqrt(deg), 0.0).astype(np.float32)
    sqdeg = np.sqrt(deg).astype(np.float32)
    dinv_col = np.ones((NCORES, 128, NT), dtype=np.float32)
    dinv_b = np.ones((NCORES, 128, NSH_PAD), dtype=np.float32)
    sqdeg_row = np.ones((NCORES, 1, NSH_PAD), dtype=ml_dtypes.bfloat16)
    for c in range(NCORES):
        d = dinv[c * NSH : (c + 1) * NSH]
        dp = np.concatenate([d, np.ones(NSH_PAD - NSH, dtype=np.float32)])
        dinv_col[c] = dp.reshape(NT, P).T
        dinv_b[c] = np.broadcast_to(dp, (128, NSH_PAD))
        s = sqdeg[c * NSH : (c + 1) * NSH]
        sp = np.concatenate([s, np.ones(NSH_PAD - NSH, dtype=np.float32)])
        sqdeg_row[c, 0] = sp.astype(ml_dtypes.bfloat16)

    # x transposed shards, hi/lo bf16
    xT_hi = np.zeros((NCORES, 128, NSH_PAD), dtype=ml_dtypes.bfloat16)
    xT_lo = np.zeros((NCORES, 128, NSH_PAD), dtype=ml_dtypes.bfloat16)
    for c in range(NCORES):
        xs = np.asarray(x[c * NSH : (c + 1) * NSH]).astype(np.float32).T
        hi, lo = _split_hi_lo(xs)
        xT_hi[c, :, :NSH] = hi
        xT_lo[c, :, :NSH] = lo

    W0h, W0l = _split_hi_lo(np.asarray(W0, dtype=np.float32))
    W1h, W1l = _split_hi_lo(np.asarray(W1, dtype=np.float32))
    Wlh, Wll = _split_hi_lo(np.asarray(Wl, dtype=np.float32))
    b0r = np.asarray(b0, dtype=np.float32).reshape(1, HID).astype(ml_dtypes.bfloat16)
    b1r = np.asarray(b1, dtype=np.float32).reshape(1, HID).astype(ml_dtypes.bfloat16)
    blb12 = np.tile(np.asarray(bl, dtype=np.float32).reshape(1, C_OUT), (128, TG))

    in_maps = []
    for c in range(NCORES):
        in_maps.append(
            {
                "xT_hi": xT_hi[c],
                "xT_lo": xT_lo[c],
                "gidx": gidx_all[c],
                "dsl": dsl_all[c],
                "dinv_col": dinv_col[c],
                "dinv_b": dinv_b[c],
                "sqdeg_row": sqdeg_row[c],
                "W0h": W0h, "W0l": W0l,
                "W1h": W1h, "W1l": W1l,
                "Wlh": Wlh, "Wll": Wll,
                "b0r": b0r, "b1r": b1r, "blb12": blb12,
            }
        )
    return in_maps, nblk_tk, tot_blocks, tot_slots


def _build_program(nblk_tk, tot_blocks, tot_slots):
    nc = bacc.Bacc(num_devices=NCORES, num_swdge_queues=NQ)
    xT_hi = nc.declare_dram_parameter("xT_hi", [128, NSH_PAD], BF16, isOutput=False)
    xT_lo = nc.declare_dram_parameter("xT_lo", [128, NSH_PAD], BF16, isOutput=False)
    gidx = nc.declare_dram_parameter("gidx", [128, tot_slots // 16], I16, isOutput=False)
    dsl = nc.declare_dram_parameter("dsl", [128, tot_blocks], BF16, isOutput=False)
    dinv_col_e = nc.declare_dram_parameter("dinv_col", [128, NT], F32, isOutput=False)
    dinv_b_e = nc.declare_dram_parameter("dinv_b", [128, NSH_PAD], F32, isOutput=False)
    sqdeg_e = nc.declare_dram_parameter("sqdeg_row", [1, NSH_PAD], BF16, isOutput=False)
    W0h = nc.declare_dram_parameter("W0h", [128, HID], BF16, isOutput=False)
    W0l = nc.declare_dram_parameter("W0l", [128, HID], BF16, isOutput=False)
    W1h = nc.declare_dram_parameter("W1h", [128, HID], BF16, isOutput=False)
    W1l = nc.declare_dram_parameter("W1l", [128, HID], BF16, isOutput=False)
    Wlh = nc.declare_dram_parameter("Wlh", [128, C_OUT], BF16, isOutput=False)
    Wll = nc.declare_dram_parameter("Wll", [128, C_OUT], BF16, isOutput=False)
    b0r_e = nc.declare_dram_parameter("b0r", [1, HID], BF16, isOutput=False)
    b1r_e = nc.declare_dram_parameter("b1r", [1, HID], BF16, isOutput=False)
    blb12_e = nc.declare_dram_parameter("blb12", [128, TG * C_OUT], F32, isOutput=False)
    out_ext = nc.declare_dram_parameter("out", [NSH, C_OUT], F32, isOutput=True)

    t1_shard = nc.dram_tensor("t1_shard", [NSH, HID], BF16)
    t2_shard = nc.dram_tensor("t2_shard", [NSH, HID], BF16)
    T1_full = nc.dram_tensor("T1_full", [N, HID], BF16, addr_space="Shared")
    T2_full = nc.dram_tensor("T2_full", [N, HID], BF16, addr_space="Shared")

    # per-(tg,k) slot counts
    sgk = np.zeros((NTG, NCHUNK), dtype=np.int64)
    for g in range(NTG):
        tiles = range(g * TG, min((g + 1) * TG, NT))
        for k in range(NCHUNK):
            sgk[g][k] = P * sum(int(nblk_tk[t][k]) for t in tiles)
    max_gk_blocks = int(sgk.max()) // P

    qctr = [0]  # SWDGE queue round-robin

    from contextlib import ExitStack
    with tile.TileContext(nc) as tc, ExitStack() as es:
        cpool = es.enter_context(tc.tile_pool(name="const", bufs=1))
        xpool = es.enter_context(tc.tile_pool(name="xp", bufs=3))
        gpool = es.enter_context(tc.tile_pool(name="gp", bufs=2))
        ipool = es.enter_context(tc.tile_pool(name="ip", bufs=2))
        dpool = es.enter_context(tc.tile_pool(name="dp", bufs=2))
        spool = es.enter_context(tc.tile_pool(name="sp", bufs=3))
        zpool = es.enter_context(tc.tile_pool(name="zp", bufs=1))
        opool = es.enter_context(tc.tile_pool(name="op", bufs=2))
        apsum = es.enter_context(tc.tile_pool(name="apsum", bufs=2, space="PSUM"))
        wpsum = es.enter_context(tc.tile_pool(name="wpsum", bufs=2, space="PSUM"))

        # ---- constants ----
        iota_t = cpool.tile([P, P], BF16, tag="iota")
        nc.gpsimd.iota(iota_t[:], pattern=[[1, P]], base=0, channel_multiplier=0,
                       allow_small_or_imprecise_dtypes=True)
        w0h_t = cpool.tile([128, HID], BF16, tag="w0h")
        w0l_t = cpool.tile([128, HID], BF16, tag="w0l")
        w1h_t = cpool.tile([128, HID], BF16, tag="w1h")
        w1l_t = cpool.tile([128, HID], BF16, tag="w1l")
        wlh_t = cpool.tile([128, C_OUT], BF16, tag="wlh")
        wll_t = cpool.tile([128, C_OUT], BF16, tag="wll")
        b0_t = cpool.tile([1, HID], BF16, tag="b0")
        b1_t = cpool.tile([1, HID], BF16, tag="b1")
        blb_t = cpool.tile([128, TG * C_OUT], F32, tag="blb")
        dinv_col = cpool.tile([128, NT], F32, tag="dcol")
        dinv_b = cpool.tile([128, NSH_PAD], F32, tag="dinvb")
        sqdeg_t = cpool.tile([1, NSH_PAD], BF16, tag="sqdeg")
        for tt, ext in [(w0h_t, W0h), (w0l_t, W0l), (w1h_t, W1h), (w1l_t, W1l),
                        (wlh_t, Wlh), (wll_t, Wll), (b0_t, b0r_e), (b1_t, b1r_e),
                        (blb_t, blb12_e), (dinv_col, dinv_col_e),
                        (dinv_b, dinv_b_e), (sqdeg_t, sqdeg_e)]:
            nc.sync.dma_start(out=tt[:], in_=ext[:, :])

        # ---- phase 1: T1 shard = dinv * (X @ W0) ----
        for t in range(NT):
            rows = min(P, NSH - t * P)
            xh = xpool.tile([128, P], BF16, tag="xh")
            xl = xpool.tile([128, P], BF16, tag="xl")
            nc.sync.dma_start(out=xh[:], in_=xT_hi[:, t * P : (t + 1) * P])
            nc.sync.dma_start(out=xl[:], in_=xT_lo[:, t * P : (t + 1) * P])
            ps = wpsum.tile([P, 512], F32, tag="wps", space="PSUM")
            nc.tensor.matmul(out=ps[:, :HID], lhsT=xh[:], rhs=w0h_t[:], start=True,
                             stop=False, skip_group_check=True)
            nc.tensor.matmul(out=ps[:, :HID], lhsT=xh[:], rhs=w0l_t[:], start=False,
                             stop=False, skip_group_check=True)
            nc.tensor.matmul(out=ps[:, :HID], lhsT=xl[:], rhs=w0h_t[:], start=False,
                             stop=True, skip_group_check=True)
            tb = opool.tile([P, HID], BF16, tag="tb")
            nc.vector.tensor_scalar(out=tb[:], in0=ps[:, :HID],
                                    scalar1=dinv_col[:, t : t + 1],
                                    scalar2=None, op0=mybir.AluOpType.mult)
            nc.sync.dma_start(out=t1_shard[t * P : t * P + rows, :], in_=tb[:rows, :])

        # ---- allgather T1 ----
        nc.gpsimd.collective_compute(
            "AllGather", mybir.AluOpType.bypass,
            replica_groups=[list(range(NCORES))],
            ins=[t1_shard[:].opt()], outs=[T1_full[:].opt()],
        )

        # ---- aggregation layers ----
        def agg_layer(T_full, layer):
            gcol = [0]
            bcol = [0]
            for g in range(NTG):
                tiles = list(range(g * TG, min((g + 1) * TG, NT)))
                ntl = len(tiles)
                gw = ntl * P            # group free width (nodes)
                goff = g * TG * P       # node offset of group start
                first_k = {}
                last_k = {}
                for t in tiles:
                    ks = [k for k in range(NCHUNK) if nblk_tk[t][k] > 0]
                    first_k[t], last_k[t] = ks[0], ks[-1]
                nbank = (ntl + 3) // 4
                banks = [apsum.tile([P, 512], F32, tag=f"agg{i}", space="PSUM",
                                    name=f"aggbank{i}")
                         for i in range(nbank)]

                def agg_ap(ti):
                    i = tiles.index(ti)
                    return banks[i // 4][:, (i % 4) * P : (i % 4 + 1) * P]

                # bias-fold: psum <- bias[feat] * sqrt(deg[node]); after the
                # dinv epilogue scale this is exactly +bias.
                bias_r = b0_t if layer == 1 else b1_t
                for i in range(nbank):
                    bw = min(512, gw - i * 512)
                    nc.tensor.matmul(
                        out=banks[i][:, :bw], lhsT=bias_r[:],
                        rhs=sqdeg_t[:, goff + i * 512 : goff + i * 512 + bw],
                        start=True, stop=False, skip_group_check=True)

                # dstslot slab for this tile group
                nb_tg = sum(int(nblk_tk[t][k]) for t in tiles for k in range(NCHUNK))
                dst_t = dpool.tile([128, nb_tg], BF16, tag="dsl")
                nc.sync.dma_start(out=dst_t[:], in_=dsl[:, bcol[0] : bcol[0] + nb_tg])

                dcol_off = 0

                def onehot_batch(c0, cnt):
                    s_t = spool.tile([P, OB, P], BF16, tag="onehot")
                    nc.vector.tensor_tensor(
                        out=s_t[:, :cnt, :],
                        in0=iota_t[:].unsqueeze(1).broadcast_to([P, cnt, P]),
                        in1=dst_t[:, c0 : c0 + cnt].unsqueeze(2)
                            .broadcast_to([P, cnt, P]),
                        op=mybir.AluOpType.is_equal,
                    )
                    return s_t

                for k in range(NCHUNK):
                    s_gk = int(sgk[g][k])
                    if s_gk == 0:
                        continue
                    kend = min((k + 1) * CH, N)
                    idxt = ipool.tile([128, max(int(sgk.max()) // 16, 16)], I16,
                                      tag="gidx")
                    nc.sync.dma_start(out=idxt[:, : s_gk // 16],
                                      in_=gidx[:, gcol[0] : gcol[0] + s_gk // 16])
                    gbuf = gpool.tile([P, max_gk_blocks, P], BF16, tag="gath")
                    nblk_gk = s_gk // P
                    nc.gpsimd.dma_gather(
                        gbuf[:, :nblk_gk, :], T_full[k * CH : kend, :],
                        idxt[:, : s_gk // 16], s_gk, s_gk, HID,
                        single_packet=False, queue_num=qctr[0] % NQ,
                    )
                    qctr[0] += 1
                    gcol[0] += s_gk // 16
                    boff = 0
                    batch = None
                    batch_c0 = -1
                    for t in tiles:
                        nb = int(nblk_tk[t][k])
                        for b in range(nb):
                            if batch is None or dcol_off - batch_c0 >= OB:
                                cnt = min(OB, nb_tg - dcol_off)
                                batch = onehot_batch(dcol_off, cnt)
                                batch_c0 = dcol_off
                            nc.tensor.matmul(
                                out=agg_ap(t), lhsT=gbuf[:, boff, :],
                                rhs=batch[:, dcol_off - batch_c0, :],
                                start=False,
                                stop=(k == last_k[t] and b == nb - 1),
                                skip_group_check=True,
                            )
                            boff += 1
                            dcol_off += 1
                bcol[0] += nb_tg

                # ---- batched epilogue: u = psum*dinv; ELU; zb (bf16) ----
                u = zpool.tile([P, TG * P], F32, tag="u")
                for i in range(nbank):
                    bw = min(512, gw - i * 512)
                    nc.vector.tensor_tensor(
                        out=u[:, i * 512 : i * 512 + bw], in0=banks[i][:, :bw],
                        in1=dinv_b[:, goff + i * 512 : goff + i * 512 + bw],
                        op=mybir.AluOpType.mult)
                mn = zpool.tile([P, TG * P], F32, tag="mn")
                nc.vector.tensor_scalar(out=mn[:, :gw], in0=u[:, :gw], scalar1=0.0,
                                        scalar2=None, op0=mybir.AluOpType.min)
                ex = zpool.tile([P, TG * P], F32, tag="ex")
                nc.scalar.activation(ex[:, :gw], mn[:, :gw],
                                     mybir.ActivationFunctionType.Exp)
                px = zpool.tile([P, TG * P], F32, tag="px")
                nc.vector.tensor_scalar(out=px[:, :gw], in0=u[:, :gw], scalar1=0.0,
                                        scalar2=None, op0=mybir.AluOpType.max)
                zs = zpool.tile([P, TG * P], F32, tag="zs")
                nc.vector.tensor_tensor(out=zs[:, :gw], in0=px[:, :gw],
                                        in1=ex[:, :gw], op=mybir.AluOpType.add)
                zb = zpool.tile([P, TG * P], BF16, tag="zb")
                nc.vector.tensor_scalar(out=zb[:, :gw], in0=zs[:, :gw], scalar1=-1.0,
                                        scalar2=None, op0=mybir.AluOpType.add)

                if layer == 1:
                    for i, t in enumerate(tiles):
                        rows = min(P, NSH - t * P)
                        ps2 = wpsum.tile([P, 512], F32, tag="wps", space="PSUM")
                        nc.tensor.matmul(out=ps2[:, :HID],
                                         lhsT=zb[:, i * P : (i + 1) * P],
                                         rhs=w1h_t[:], start=True, stop=False,
                                         skip_group_check=True)
                        nc.tensor.matmul(out=ps2[:, :HID],
                                         lhsT=zb[:, i * P : (i + 1) * P],
                                         rhs=w1l_t[:], start=False, stop=True,
                                         skip_group_check=True)
                        t2b = opool.tile([P, HID], BF16, tag="tb")
                        nc.vector.tensor_scalar(out=t2b[:], in0=ps2[:, :HID],
                                                scalar1=dinv_col[:, t : t + 1],
                                                scalar2=None,
                                                op0=mybir.AluOpType.mult)
                        nc.sync.dma_start(out=t2_shard[t * P : t * P + rows, :],
                                          in_=t2b[:rows, :])
                else:
                    cls = wpsum.tile([P, 512], F32, tag="wps", space="PSUM")
                    for i, t in enumerate(tiles):
                        nc.tensor.matmul(out=cls[:, i * C_OUT : (i + 1) * C_OUT],
                                         lhsT=zb[:, i * P : (i + 1) * P],
                                         rhs=wlh_t[:], start=True, stop=False,
                                         skip_group_check=True)
                        nc.tensor.matmul(out=cls[:, i * C_OUT : (i + 1) * C_OUT],
                                         lhsT=zb[:, i * P : (i + 1) * P],
                                         rhs=wll_t[:], start=False, stop=True,
                                         skip_group_check=True)
                    cw = ntl * C_OUT
                    lg = opool.tile([P, TG * C_OUT], F32, tag="lg")
                    nc.vector.tensor_tensor(out=lg[:, :cw], in0=cls[:, :cw],
                                            in1=blb_t[:, :cw],
                                            op=mybir.AluOpType.add)
                    ex2 = opool.tile([P, TG * C_OUT], F32, tag="ex2")
                    nc.scalar.activation(ex2[:, :cw], lg[:, :cw],
                                         mybir.ActivationFunctionType.Exp)
                    sm = opool.tile([P, TG], F32, tag="sm")
                    nc.vector.tensor_reduce(
                        out=sm[:, :ntl],
                        in_=ex2[:, :cw].rearrange("p (t c) -> p t c", c=C_OUT),
                        axis=mybir.AxisListType.X, op=mybir.AluOpType.add)
                    ln = opool.tile([P, TG], F32, tag="ln")
                    nc.scalar.activation(ln[:, :ntl], sm[:, :ntl],
                                         mybir.ActivationFunctionType.Ln)
                    res = opool.tile([P, TG * C_OUT], F32, tag="res")
                    nc.vector.tensor_tensor(
                        out=res[:, :cw].rearrange("p (t c) -> p t c", c=C_OUT),
                        in0=lg[:, :cw].rearrange("p (t c) -> p t c", c=C_OUT),
                        in1=ln[:, :ntl].unsqueeze(2).broadcast_to([P, ntl, C_OUT]),
                        op=mybir.AluOpType.subtract)
                    for i, t in enumerate(tiles):
                        rows = min(P, NSH - t * P)
                        nc.sync.dma_start(
                            out=out_ext[t * P : t * P + rows, :],
                            in_=res[:rows, i * C_OUT : (i + 1) * C_OUT])

        agg_layer(T1_full, 1)
        nc.gpsimd.collective_compute(
            "AllGather", mybir.AluOpType.bypass,
            replica_groups=[list(range(NCORES))],
            ins=[t2_shard[:].opt()], outs=[T2_full[:].opt()],
        )
        agg_layer(T2_full, 2)

    nc.finalize()
    return nc


_CACHE = {}


def kernel(**inputs):
    in_maps, nblk_tk, tot_blocks, tot_slots = _prep_host(
        inputs["x"], inputs["edge_index"], inputs["W0"], inputs["b0"],
        inputs["W1"], inputs["b1"], inputs["Wl"], inputs["bl"])
    key = (tot_blocks, tot_slots, nblk_tk.tobytes())
    if key not in _CACHE:
        _CACHE[key] = _build_program(nblk_tk, tot_blocks, tot_slots)
    nc = _CACHE[key]
    trace = bool(int(__import__("os").environ.get("KERNEL_TRACE", "0")))
    res = run_bass_kernel_spmd(nc, in_maps, list(range(NCORES)), trace=trace)
    kernel.last_results = res
    out = np.concatenate([res.results[c]["out"] for c in range(NCORES)], axis=0)
    return out.astype(np.float32)


# revision 6
# speedup vs baseline: 1.4842x; 1.2381x over previous
"""2-layer GCN + classifier on 8 Trainium2 NeuronCores.

Strategy (graph/data parallel):
- Nodes sharded 8 ways by contiguous range (12500/core). Edges partitioned by
  dst shard on host, grouped by (dst-tile of 128, src-chunk of 32768), padded
  to 128-edge blocks. Self-loops appended as ordinary edges.
- Per GCN layer: each core builds its shard of the gather table
  T = dinv * (Z @ W) (bf16), AllGather -> full table in every core's HBM.
  Aggregation: dma_gather rows by src (int16 chunk-local idx) spread over the
  4 SWDGE queues (overlaps descriptor generation with drains), one-hot
  dst-slot matrices built in wide batches via broadcast-AP is_equal,
  segment-sum via PSUM-accumulated bf16 matmuls. Bias (pre-scaled by
  sqrt(deg)) seeds the PSUM banks via a rank-1 matmul so the epilogue is just
  dinv-scale + ELU, batched per 12-tile group.
- Classifier + log_softmax computed per tile group; host concatenates the 8
  output shards.
"""
import sys

sys.path.insert(0, "/opt/trn_rl_repo")

import numpy as np
import ml_dtypes

import concourse.bacc as bacc
import concourse.tile as tile
from concourse import mybir
from concourse.bass_utils import run_bass_kernel_spmd

# ---------------- problem constants (hardcoded per task statement) ----------
N = 100000
E = 1600000
F_IN = 128
HID = 128
C_OUT = 40
NCORES = 8
NSH = N // NCORES          # 12500 nodes per core
P = 128
NT = (NSH + P - 1) // P    # 98 dst tiles per core (last has 84 rows)
NSH_PAD = NT * P           # 12544
CH = 32768                 # gather chunk rows (int16 idx limit)
NCHUNK = (N + CH - 1) // CH  # 4
TG = 12                    # dst tiles per tile-group (PSUM: 3 banks x 2 bufs)
NTG = (NT + TG - 1) // TG  # 9 tile groups
OB = 24                    # one-hot build batch (blocks per DVE op)
NQ = 4                     # SWDGE queues for gathers

F32 = mybir.dt.float32
BF16 = mybir.dt.bfloat16
I16 = mybir.dt.int16


def _split_hi_lo(w):
    hi = w.astype(ml_dtypes.bfloat16)
    lo = (w - hi.astype(np.float32)).astype(ml_dtypes.bfloat16)
    return hi, lo


def _prep_host(x, edge_index, W0, b0, W1, b1, Wl, bl):
    """Shard + reorder edges; build all per-core device input arrays."""
    src = np.asarray(edge_index[0]).astype(np.int64)
    dst = np.asarray(edge_index[1]).astype(np.int64)
    loop = np.arange(N, dtype=np.int64)
    src2 = np.concatenate([src, loop])
    dst2 = np.concatenate([dst, loop])
    deg = np.bincount(dst2, minlength=N).astype(np.float32)  # = indeg + 1

    # ---- per-core edge grouping by (dst_tile, src_chunk) ----
    counts = np.zeros((NCORES, NT, NCHUNK), dtype=np.int64)
    grouped = []  # per core: (sorted_src, dst_slot, cnt)
    core_of = dst2 // NSH
    for c in range(NCORES):
        sel = core_of == c
        es = src2[sel]
        ed = dst2[sel] - c * NSH
        tile_id = ed // P
        chunk_id = es // CH
        key = tile_id * NCHUNK + chunk_id
        order = np.argsort(key, kind="stable")
        es, ed, key = es[order], ed[order], key[order]
        cnt = np.bincount(key, minlength=NT * NCHUNK).reshape(NT, NCHUNK)
        counts[c] = cnt
        grouped.append((es, ed % P, cnt))

    nblk_tk = np.ceil(counts.max(axis=0) / P).astype(np.int64)  # [NT, NCHUNK]

    # flatten order: for tg: for k: for t in tg: blocks
    tot_blocks = 0
    for g in range(NTG):
        tiles = range(g * TG, min((g + 1) * TG, NT))
        for k in range(NCHUNK):
            for t in tiles:
                tot_blocks += int(nblk_tk[t][k])
    tot_slots = tot_blocks * P

    gidx_all = np.zeros((NCORES, 128, tot_slots // 16), dtype=np.int16)
    dsl_all = np.full((NCORES, 128, tot_blocks), -1.0, dtype=ml_dtypes.bfloat16)

    for c in range(NCORES):
        es, slots, cnt = grouped[c]
        starts = np.zeros(NT * NCHUNK + 1, dtype=np.int64)
        np.cumsum(cnt.reshape(-1), out=starts[1:])
        gcol = 0  # gidx column offset (16-wide units)
        bcol = 0  # dstslot block column offset
        for g in range(NTG):
            tiles = range(g * TG, min((g + 1) * TG, NT))
            for k in range(NCHUNK):
                for t in tiles:
                    nb = int(nblk_tk[t][k])
                    if nb == 0:
                        continue
                    a, b = starts[t * NCHUNK + k], starts[t * NCHUNK + k + 1]
                    n = b - a
                    pad = nb * P - n
                    sl = np.concatenate(
                        [es[a:b] - k * CH, np.zeros(pad, dtype=np.int64)]
                    ).astype(np.int16)
                    ds = np.concatenate(
                        [slots[a:b], np.full(pad, -1, dtype=np.int64)]
                    ).astype(ml_dtypes.bfloat16)
                    # wrap idx: slot i -> [i%16, i//16]
                    gidx_all[c, :16, gcol : gcol + nb * 8] = sl.reshape(-1, 16).T
                    dsl_all[c, :, bcol : bcol + nb] = ds.reshape(nb, P).T
                    gcol += nb * 8
                    bcol += nb
        gidx_all[c] = np.tile(gidx_all[c, :16], (8, 1))

    # degree-derived tables (host-precomputed)
    dinv = np.where(deg > 0, 1.0 / np.sqrt(deg), 0.0).astype(np.float32)
    sqdeg = np.sqrt(deg).astype(np.float32)
    dinv_col = np.ones((NCORES, 128, NT), dtype=np.float32)
    dinv_b = np.ones((NCORES, 128, NSH_PAD), dtype=np.float32)
    sqdeg_row = np.ones((NCORES, 1, NSH_PAD), dtype=ml_dtypes.bfloat16)
    for c in range(NCORES):
        d = dinv[c * NSH : (c + 1) * NSH]
        dp = np.concatenate([d, np.ones(NSH_PAD - NSH, dtype=np.float32)])
        dinv_col[c] = dp.reshape(NT, P).T
        dinv_b[c] = np.broadcast_to(dp, (128, NSH_PAD))
        s = sqdeg[c * NSH : (c + 1) * NSH]
        sp = np.concatenate([s, np.ones(NSH_PAD - NSH, dtype=np.float32)])
        sqdeg_row[c, 0] = sp.astype(ml_dtypes.bfloat16)

    # x transposed shards, hi/lo bf16
    xT_hi = np.zeros((NCORES, 128, NSH_PAD), dtype=ml_dtypes.bfloat16)
    xT_lo = np.zeros((NCORES, 128, NSH_PAD), dtype=ml_dtypes.bfloat16)
    for c in range(NCORES):
        xs = np.asarray(x[c * NSH : (c + 1) * NSH]).astype(np.float32).T
        hi, lo = _split_hi_lo(xs)
        xT_hi[c, :, :NSH] = hi
        xT_lo[c, :, :NSH] = lo

    W0h, W0l = _split_hi_lo(np.asarray(W0, dtype=np.float32))
    W1h, W1l = _split_hi_lo(np.asarray(W1, dtype=np.float32))
    Wlh, Wll = _split_hi_lo(np.asarray(Wl, dtype=np.float32))
    b0r = np.asarray(b0, dtype=np.float32).reshape(1, HID).astype(ml_dtypes.bfloat16)
    b1r = np.asarray(b1, dtype=np.float32).reshape(1, HID).astype(ml_dtypes.bfloat16)
    blb12 = np.tile(np.asarray(bl, dtype=np.float32).reshape(1, C_OUT), (128, TG))

    in_maps = []
    for c in range(NCORES):
        in_maps.append(
            {
                "xT_hi": xT_hi[c],
                "xT_lo": xT_lo[c],
                "gidx": gidx_all[c],
                "dsl": dsl_all[c],
                "dinv_col": dinv_col[c],
                "dinv_b": dinv_b[c],
                "sqdeg_row": sqdeg_row[c],
                "W0h": W0h, "W0l": W0l,
                "W1h": W1h, "W1l": W1l,
                "Wlh": Wlh, "Wll": Wll,
                "b0r": b0r, "b1r": b1r, "blb12": blb12,
            }
        )
    return in_maps, nblk_tk, tot_blocks, tot_slots


def _build_program(nblk_tk, tot_blocks, tot_slots):
    nc = bacc.Bacc(num_devices=NCORES, num_swdge_queues=NQ)
    xT_hi = nc.declare_dram_parameter("xT_hi", [128, NSH_PAD], BF16, isOutput=False)
    xT_lo = nc.declare_dram_parameter("xT_lo", [128, NSH_PAD], BF16, isOutput=False)
    gidx = nc.declare_dram_parameter("gidx", [128, tot_slots // 16], I16, isOutput=False)
    dsl = nc.declare_dram_parameter("dsl", [128, tot_blocks], BF16, isOutput=False)
    dinv_col_e = nc.declare_dram_parameter("dinv_col", [128, NT], F32, isOutput=False)
    dinv_b_e = nc.declare_dram_parameter("dinv_b", [128, NSH_PAD], F32, isOutput=False)
    sqdeg_e = nc.declare_dram_parameter("sqdeg_row", [1, NSH_PAD], BF16, isOutput=False)
    W0h = nc.declare_dram_parameter("W0h", [128, HID], BF16, isOutput=False)
    W0l = nc.declare_dram_parameter("W0l", [128, HID], BF16, isOutput=False)
    W1h = nc.declare_dram_parameter("W1h", [128, HID], BF16, isOutput=False)
    W1l = nc.declare_dram_parameter("W1l", [128, HID], BF16, isOutput=False)
    Wlh = nc.declare_dram_parameter("Wlh", [128, C_OUT], BF16, isOutput=False)
    Wll = nc.declare_dram_parameter("Wll", [128, C_OUT], BF16, isOutput=False)
    b0r_e = nc.declare_dram_parameter("b0r", [1, HID], BF16, isOutput=False)
    b1r_e = nc.declare_dram_parameter("b1r", [1, HID], BF16, isOutput=False)
    blb12_e = nc.declare_dram_parameter("blb12", [128, TG * C_OUT], F32, isOutput=False)
    out_ext = nc.declare_dram_parameter("out", [NSH, C_OUT], F32, isOutput=True)

    t1_shard = nc.dram_tensor("t1_shard", [NSH, HID], BF16)
    t2_shard = nc.dram_tensor("t2_shard", [NSH, HID], BF16)
    T1_full = nc.dram_tensor("T1_full", [N, HID], BF16, addr_space="Shared")
    T2_full = nc.dram_tensor("T2_full", [N, HID], BF16, addr_space="Shared")

    # per-(tg,k) slot counts
    sgk = np.zeros((NTG, NCHUNK), dtype=np.int64)
    for g in range(NTG):
        tiles = range(g * TG, min((g + 1) * TG, NT))
        for k in range(NCHUNK):
            sgk[g][k] = P * sum(int(nblk_tk[t][k]) for t in tiles)
    max_gk_blocks = int(sgk.max()) // P

    qctr = [0]  # SWDGE queue round-robin

    from contextlib import ExitStack
    with tile.TileContext(nc) as tc, ExitStack() as es:
        cpool = es.enter_context(tc.tile_pool(name="const", bufs=1))
        xpool = es.enter_context(tc.tile_pool(name="xp", bufs=3))
        gpool = es.enter_context(tc.tile_pool(name="gp", bufs=2))
        ipool = es.enter_context(tc.tile_pool(name="ip", bufs=2))
        dpool = es.enter_context(tc.tile_pool(name="dp", bufs=2))
        spool = es.enter_context(tc.tile_pool(name="sp", bufs=3))
        zpool = es.enter_context(tc.tile_pool(name="zp", bufs=1))
        opool = es.enter_context(tc.tile_pool(name="op", bufs=2))
        apsum = es.enter_context(tc.tile_pool(name="apsum", bufs=2, space="PSUM"))
        wpsum = es.enter_context(tc.tile_pool(name="wpsum", bufs=2, space="PSUM"))

        # ---- constants ----
        iota_t = cpool.tile([P, P], BF16, tag="iota")
        nc.gpsimd.iota(iota_t[:], pattern=[[1, P]], base=0, channel_multiplier=0,
                       allow_small_or_imprecise_dtypes=True)
        w0h_t = cpool.tile([128, HID], BF16, tag="w0h")
        w0l_t = cpool.tile([128, HID], BF16, tag="w0l")
        w1h_t = cpool.tile([128, HID], BF16, tag="w1h")
        w1l_t = cpool.tile([128, HID], BF16, tag="w1l")
        wlh_t = cpool.tile([128, C_OUT], BF16, tag="wlh")
        wll_t = cpool.tile([128, C_OUT], BF16, tag="wll")
        b0_t = cpool.tile([1, HID], BF16, tag="b0")
        b1_t = cpool.tile([1, HID], BF16, tag="b1")
        blb_t = cpool.tile([128, TG * C_OUT], F32, tag="blb")
        dinv_col = cpool.tile([128, NT], F32, tag="dcol")
        dinv_b = cpool.tile([128, NSH_PAD], F32, tag="dinvb")
        sqdeg_t = cpool.tile([1, NSH_PAD], BF16, tag="sqdeg")
        for tt, ext in [(w0h_t, W0h), (w0l_t, W0l), (w1h_t, W1h), (w1l_t, W1l),
                        (wlh_t, Wlh), (wll_t, Wll), (b0_t, b0r_e), (b1_t, b1r_e),
                        (blb_t, blb12_e), (dinv_col, dinv_col_e),
                        (dinv_b, dinv_b_e), (sqdeg_t, sqdeg_e)]:
            nc.sync.dma_start(out=tt[:], in_=ext[:, :])

        # ---- phase 1: T1 shard = dinv * (X @ W0) ----
        for t in range(NT):
            rows = min(P, NSH - t * P)
            xh = xpool.tile([128, P], BF16, tag="xh")
            xl = xpool.tile([128, P], BF16, tag="xl")
            nc.sync.dma_start(out=xh[:], in_=xT_hi[:, t * P : (t + 1) * P])
            nc.sync.dma_start(out=xl[:], in_=xT_lo[:, t * P : (t + 1) * P])
            ps = wpsum.tile([P, 512], F32, tag="wps", space="PSUM")
            nc.tensor.matmul(out=ps[:, :HID], lhsT=xh[:], rhs=w0h_t[:], start=True,
                             stop=False, skip_group_check=True)
            nc.tensor.matmul(out=ps[:, :HID], lhsT=xh[:], rhs=w0l_t[:], start=False,
                             stop=False, skip_group_check=True)
            nc.tensor.matmul(out=ps[:, :HID], lhsT=xl[:], rhs=w0h_t[:], start=False,
                             stop=True, skip_group_check=True)
            tb = opool.tile([P, HID], BF16, tag="tb")
            nc.vector.tensor_scalar(out=tb[:], in0=ps[:, :HID],
                                    scalar1=dinv_col[:, t : t + 1],
                                    scalar2=None, op0=mybir.AluOpType.mult)
            nc.sync.dma_start(out=t1_shard[t * P : t * P + rows, :], in_=tb[:rows, :])

        # ---- allgather T1 ----
        nc.gpsimd.collective_compute(
            "AllGather", mybir.AluOpType.bypass,
            replica_groups=[list(range(NCORES))],
            ins=[t1_shard[:].opt()], outs=[T1_full[:].opt()],
        )

        # ---- aggregation layers ----
        def agg_layer(T_full, layer):
            gcol = [0]
            bcol = [0]

            def emit_agg(g):
                tiles = list(range(g * TG, min((g + 1) * TG, NT)))
                ntl = len(tiles)
                gw = ntl * P            # group free width (nodes)
                goff = g * TG * P       # node offset of group start
                first_k = {}
                last_k = {}
                for t in tiles:
                    ks = [k for k in range(NCHUNK) if nblk_tk[t][k] > 0]
                    first_k[t], last_k[t] = ks[0], ks[-1]
                nbank = (ntl + 3) // 4
                banks = [apsum.tile([P, 512], F32, tag=f"agg{i}", space="PSUM",
                                    name=f"aggbank{i}")
                         for i in range(nbank)]

                def agg_ap(ti):
                    i = tiles.index(ti)
                    return banks[i // 4][:, (i % 4) * P : (i % 4 + 1) * P]

                # bias-fold: psum <- bias[feat] * sqrt(deg[node]); after the
                # dinv epilogue scale this is exactly +bias.
                bias_r = b0_t if layer == 1 else b1_t
                for i in range(nbank):
                    bw = min(512, gw - i * 512)
                    nc.tensor.matmul(
                        out=banks[i][:, :bw], lhsT=bias_r[:],
                        rhs=sqdeg_t[:, goff + i * 512 : goff + i * 512 + bw],
                        start=True, stop=False, skip_group_check=True)

                # dstslot slab for this tile group
                nb_tg = sum(int(nblk_tk[t][k]) for t in tiles for k in range(NCHUNK))
                dst_t = dpool.tile([128, nb_tg], BF16, tag="dsl")
                nc.sync.dma_start(out=dst_t[:], in_=dsl[:, bcol[0] : bcol[0] + nb_tg])

                dcol_off = 0

                def onehot_batch(c0, cnt):
                    s_t = spool.tile([P, OB, P], BF16, tag="onehot")
                    nc.vector.tensor_tensor(
                        out=s_t[:, :cnt, :],
                        in0=iota_t[:].unsqueeze(1).broadcast_to([P, cnt, P]),
                        in1=dst_t[:, c0 : c0 + cnt].unsqueeze(2)
                            .broadcast_to([P, cnt, P]),
                        op=mybir.AluOpType.is_equal,
                    )
                    return s_t

                for k in range(NCHUNK):
                    s_gk = int(sgk[g][k])
                    if s_gk == 0:
                        continue
                    kend = min((k + 1) * CH, N)
                    idxt = ipool.tile([128, max(int(sgk.max()) // 16, 16)], I16,
                                      tag="gidx")
                    nc.sync.dma_start(out=idxt[:, : s_gk // 16],
                                      in_=gidx[:, gcol[0] : gcol[0] + s_gk // 16])
                    gbuf = gpool.tile([P, max_gk_blocks, P], BF16, tag="gath")
                    nblk_gk = s_gk // P
                    # split into two half-gathers on different SWDGE queues
                    h = (nblk_gk + 1) // 2
                    for b0, b1 in ((0, h), (h, nblk_gk)):
                        if b1 <= b0:
                            continue
                        s_h = (b1 - b0) * P
                        nc.gpsimd.dma_gather(
                            gbuf[:, b0:b1, :], T_full[k * CH : kend, :],
                            idxt[:, b0 * 8 : b0 * 8 + s_h // 16], s_h, s_h, HID,
                            single_packet=False, queue_num=qctr[0] % NQ,
                        )
                        qctr[0] += 1
                    gcol[0] += s_gk // 16
                    boff = 0
                    batch = None
                    batch_c0 = -1
                    for t in tiles:
                        nb = int(nblk_tk[t][k])
                        for b in range(nb):
                            if batch is None or dcol_off - batch_c0 >= OB:
                                cnt = min(OB, nb_tg - dcol_off)
                                batch = onehot_batch(dcol_off, cnt)
                                batch_c0 = dcol_off
                            nc.tensor.matmul(
                                out=agg_ap(t), lhsT=gbuf[:, boff, :],
                                rhs=batch[:, dcol_off - batch_c0, :],
                                start=False,
                                stop=(k == last_k[t] and b == nb - 1),
                                skip_group_check=True,
                            )
                            boff += 1
                            dcol_off += 1
                bcol[0] += nb_tg
                return banks, tiles, ntl, gw, goff

            def emit_epilogue(ctx):
                banks, tiles, ntl, gw, goff = ctx
                nbank = (ntl + 3) // 4
                # ---- batched epilogue: u = psum*dinv; ELU; zb (bf16) ----
                u = zpool.tile([P, TG * P], F32, tag="u")
                for i in range(nbank):
                    bw = min(512, gw - i * 512)
                    nc.vector.tensor_tensor(
                        out=u[:, i * 512 : i * 512 + bw], in0=banks[i][:, :bw],
                        in1=dinv_b[:, goff + i * 512 : goff + i * 512 + bw],
                        op=mybir.AluOpType.mult)
                mn = zpool.tile([P, TG * P], F32, tag="mn")
                nc.vector.tensor_scalar(out=mn[:, :gw], in0=u[:, :gw], scalar1=0.0,
                                        scalar2=None, op0=mybir.AluOpType.min)
                ex = zpool.tile([P, TG * P], F32, tag="ex")
                nc.scalar.activation(ex[:, :gw], mn[:, :gw],
                                     mybir.ActivationFunctionType.Exp)
                px = zpool.tile([P, TG * P], F32, tag="px")
                nc.vector.tensor_scalar(out=px[:, :gw], in0=u[:, :gw], scalar1=0.0,
                                        scalar2=None, op0=mybir.AluOpType.max)
                zs = zpool.tile([P, TG * P], F32, tag="zs")
                nc.vector.tensor_tensor(out=zs[:, :gw], in0=px[:, :gw],
                                        in1=ex[:, :gw], op=mybir.AluOpType.add)
                zb = zpool.tile([P, TG * P], BF16, tag="zb")
                nc.vector.tensor_scalar(out=zb[:, :gw], in0=zs[:, :gw], scalar1=-1.0,
                                        scalar2=None, op0=mybir.AluOpType.add)

                if layer == 1:
                    for i, t in enumerate(tiles):
                        rows = min(P, NSH - t * P)
                        ps2 = wpsum.tile([P, 512], F32, tag="wps", space="PSUM")
                        nc.tensor.matmul(out=ps2[:, :HID],
                                         lhsT=zb[:, i * P : (i + 1) * P],
                                         rhs=w1h_t[:], start=True, stop=False,
                                         skip_group_check=True)
                        nc.tensor.matmul(out=ps2[:, :HID],
                                         lhsT=zb[:, i * P : (i + 1) * P],
                                         rhs=w1l_t[:], start=False, stop=True,
                                         skip_group_check=True)
                        t2b = opool.tile([P, HID], BF16, tag="tb")
                        nc.vector.tensor_scalar(out=t2b[:], in0=ps2[:, :HID],
                                                scalar1=dinv_col[:, t : t + 1],
                                                scalar2=None,
                                                op0=mybir.AluOpType.mult)
                        nc.sync.dma_start(out=t2_shard[t * P : t * P + rows, :],
                                          in_=t2b[:rows, :])
                else:
                    cls = wpsum.tile([P, 512], F32, tag="wps", space="PSUM")
                    for i, t in enumerate(tiles):
                        nc.tensor.matmul(out=cls[:, i * C_OUT : (i + 1) * C_OUT],
                                         lhsT=zb[:, i * P : (i + 1) * P],
                                         rhs=wlh_t[:], start=True, stop=False,
                                         skip_group_check=True)
                        nc.tensor.matmul(out=cls[:, i * C_OUT : (i + 1) * C_OUT],
                                         lhsT=zb[:, i * P : (i + 1) * P],
                                         rhs=wll_t[:], start=False, stop=True,
                                         skip_group_check=True)
                    cw = ntl * C_OUT
                    lg = opool.tile([P, TG * C_OUT], F32, tag="lg")
                    nc.vector.tensor_tensor(out=lg[:, :cw], in0=cls[:, :cw],
                                            in1=blb_t[:, :cw],
                                            op=mybir.AluOpType.add)
                    ex2 = opool.tile([P, TG * C_OUT], F32, tag="ex2")
                    nc.scalar.activation(ex2[:, :cw], lg[:, :cw],
                                         mybir.ActivationFunctionType.Exp)
                    sm = opool.tile([P, TG], F32, tag="sm")
                    nc.vector.tensor_reduce(
                        out=sm[:, :ntl],
                        in_=ex2[:, :cw].rearrange("p (t c) -> p t c", c=C_OUT),
                        axis=mybir.AxisListType.X, op=mybir.AluOpType.add)
                    ln = opool.tile([P, TG], F32, tag="ln")
                    nc.scalar.activation(ln[:, :ntl], sm[:, :ntl],
                                         mybir.ActivationFunctionType.Ln)
                    res = opool.tile([P, TG * C_OUT], F32, tag="res")
                    nc.vector.tensor_tensor(
                        out=res[:, :cw].rearrange("p (t c) -> p t c", c=C_OUT),
                        in0=lg[:, :cw].rearrange("p (t c) -> p t c", c=C_OUT),
                        in1=ln[:, :ntl].unsqueeze(2).broadcast_to([P, ntl, C_OUT]),
                        op=mybir.AluOpType.subtract)
                    for i, t in enumerate(tiles):
                        rows = min(P, NSH - t * P)
                        nc.sync.dma_start(
                            out=out_ext[t * P : t * P + rows, :],
                            in_=res[:rows, i * C_OUT : (i + 1) * C_OUT])

            # software pipeline: epilogue of group g runs one group behind
            # the aggregation stream so Vector never head-of-line blocks.
            pending = []
            for g in range(NTG):
                pending.append(emit_agg(g))
                if len(pending) == 2:
                    emit_epilogue(pending.pop(0))
            while pending:
                emit_epilogue(pending.pop(0))

        agg_layer(T1_full, 1)
        nc.gpsimd.collective_compute(
            "AllGather", mybir.AluOpType.bypass,
            replica_groups=[list(range(NCORES))],
            ins=[t2_shard[:].opt()], outs=[T2_full[:].opt()],
        )
        agg_layer(T2_full, 2)

    nc.finalize()
    return nc


_CACHE = {}


def kernel(**inputs):
    in_maps, nblk_tk, tot_blocks, tot_slots = _prep_host(
        inputs["x"], inputs["edge_index"], inputs["W0"], inputs["b0"],
        inputs["W1"], inputs["b1"], inputs["Wl"], inputs["bl"])
    key = (tot_blocks, tot_slots, nblk_tk.tobytes())
    if key not in _CACHE:
        _CACHE[key] = _build_program(nblk_tk, tot_blocks, tot_slots)
    nc = _CACHE[key]
    trace = bool(int(__import__("os").environ.get("KERNEL_TRACE", "0")))
    res = run_bass_kernel_spmd(nc, in_maps, list(range(NCORES)), trace=trace)
    kernel.last_results = res
    out = np.concatenate([res.results[c]["out"] for c in range(NCORES)], axis=0)
    return out.astype(np.float32)


# revision 7
# speedup vs baseline: 1.6070x; 1.0827x over previous
"""2-layer GCN + classifier on 8 Trainium2 NeuronCores.

Strategy (graph/data parallel):
- Nodes sharded 8 ways by contiguous range (12500/core). Edges partitioned by
  dst shard on host, grouped by (dst-tile of 128, src-chunk of 32768), padded
  to 128-edge blocks. Self-loops appended as ordinary edges.
- Per GCN layer: each core builds its shard of the gather table
  T = dinv * (Z @ W) (bf16), AllGather -> full table in every core's HBM.
  Aggregation: dma_gather rows by src (int16 chunk-local idx) spread over the
  4 SWDGE queues (overlaps descriptor generation with drains), one-hot
  dst-slot matrices built in wide batches via broadcast-AP is_equal,
  segment-sum via PSUM-accumulated bf16 matmuls. Bias (pre-scaled by
  sqrt(deg)) seeds the PSUM banks via a rank-1 matmul so the epilogue is just
  dinv-scale + ELU, batched per 12-tile group.
- Classifier + log_softmax computed per tile group; host concatenates the 8
  output shards.
"""
import sys

sys.path.insert(0, "/opt/trn_rl_repo")

import numpy as np
import ml_dtypes

import concourse.bacc as bacc
import concourse.tile as tile
from concourse import mybir
from concourse.bass_utils import run_bass_kernel_spmd

# ---------------- problem constants (hardcoded per task statement) ----------
N = 100000
E = 1600000
F_IN = 128
HID = 128
C_OUT = 40
NCORES = 8
NSH = N // NCORES          # 12500 nodes per core
P = 128
NT = (NSH + P - 1) // P    # 98 dst tiles per core (last has 84 rows)
NSH_PAD = NT * P           # 12544
CH = 32768                 # gather chunk rows (int16 idx limit)
NCHUNK = (N + CH - 1) // CH  # 4
TG = 12                    # dst tiles per tile-group (PSUM: 3 banks x 2 bufs)
NTG = (NT + TG - 1) // TG  # 9 tile groups
OB = 24                    # one-hot build batch (blocks per DVE op)
NQ = 4                     # SWDGE queues for gathers

F32 = mybir.dt.float32
BF16 = mybir.dt.bfloat16
I16 = mybir.dt.int16


def _split_hi_lo(w):
    hi = w.astype(ml_dtypes.bfloat16)
    lo = (w - hi.astype(np.float32)).astype(ml_dtypes.bfloat16)
    return hi, lo


def _prep_host(x, edge_index, W0, b0, W1, b1, Wl, bl):
    """Shard + reorder edges; build all per-core device input arrays."""
    src = np.asarray(edge_index[0]).astype(np.int64)
    dst = np.asarray(edge_index[1]).astype(np.int64)
    loop = np.arange(N, dtype=np.int64)
    src2 = np.concatenate([src, loop])
    dst2 = np.concatenate([dst, loop])
    deg = np.bincount(dst2, minlength=N).astype(np.float32)  # = indeg + 1

    # ---- per-core edge grouping by (dst_tile, src_chunk) ----
    counts = np.zeros((NCORES, NT, NCHUNK), dtype=np.int64)
    grouped = []  # per core: (sorted_src, dst_slot, cnt)
    core_of = dst2 // NSH
    for c in range(NCORES):
        sel = core_of == c
        es = src2[sel]
        ed = dst2[sel] - c * NSH
        tile_id = ed // P
        chunk_id = es // CH
        key = tile_id * NCHUNK + chunk_id
        order = np.argsort(key, kind="stable")
        es, ed, key = es[order], ed[order], key[order]
        cnt = np.bincount(key, minlength=NT * NCHUNK).reshape(NT, NCHUNK)
        counts[c] = cnt
        grouped.append((es, ed % P, cnt))

    nblk_tk = np.ceil(counts.max(axis=0) / P).astype(np.int64)  # [NT, NCHUNK]

    # flatten order: for tg: for k: for t in tg: blocks
    tot_blocks = 0
    for g in range(NTG):
        tiles = range(g * TG, min((g + 1) * TG, NT))
        for k in range(NCHUNK):
            for t in tiles:
                tot_blocks += int(nblk_tk[t][k])
    tot_slots = tot_blocks * P

    gidx_all = np.zeros((NCORES, 128, tot_slots // 16), dtype=np.int16)
    dsl_all = np.full((NCORES, 128, tot_blocks), -1.0, dtype=ml_dtypes.bfloat16)

    for c in range(NCORES):
        es, slots, cnt = grouped[c]
        starts = np.zeros(NT * NCHUNK + 1, dtype=np.int64)
        np.cumsum(cnt.reshape(-1), out=starts[1:])
        gcol = 0  # gidx column offset (16-wide units)
        bcol = 0  # dstslot block column offset
        for g in range(NTG):
            tiles = range(g * TG, min((g + 1) * TG, NT))
            for k in range(NCHUNK):
                for t in tiles:
                    nb = int(nblk_tk[t][k])
                    if nb == 0:
                        continue
                    a, b = starts[t * NCHUNK + k], starts[t * NCHUNK + k + 1]
                    n = b - a
                    pad = nb * P - n
                    sl = np.concatenate(
                        [es[a:b] - k * CH, np.zeros(pad, dtype=np.int64)]
                    ).astype(np.int16)
                    ds = np.concatenate(
                        [slots[a:b], np.full(pad, -1, dtype=np.int64)]
                    ).astype(ml_dtypes.bfloat16)
                    # wrap idx: slot i -> [i%16, i//16]
                    gidx_all[c, :16, gcol : gcol + nb * 8] = sl.reshape(-1, 16).T
                    dsl_all[c, :, bcol : bcol + nb] = ds.reshape(nb, P).T
                    gcol += nb * 8
                    bcol += nb
        gidx_all[c] = np.tile(gidx_all[c, :16], (8, 1))

    # degree-derived tables (host-precomputed)
    dinv = np.where(deg > 0, 1.0 / np.sqrt(deg), 0.0).astype(np.float32)
    sqdeg = np.sqrt(deg).astype(np.float32)
    dinv_col = np.ones((NCORES, 128, NT), dtype=np.float32)
    dinv_b = np.ones((NCORES, 128, NSH_PAD), dtype=ml_dtypes.bfloat16)
    sqdeg_row = np.ones((NCORES, 1, NSH_PAD), dtype=ml_dtypes.bfloat16)
    for c in range(NCORES):
        d = dinv[c * NSH : (c + 1) * NSH]
        dp = np.concatenate([d, np.ones(NSH_PAD - NSH, dtype=np.float32)])
        dinv_col[c] = dp.reshape(NT, P).T
        dinv_b[c] = np.broadcast_to(dp.astype(ml_dtypes.bfloat16), (128, NSH_PAD))
        s = sqdeg[c * NSH : (c + 1) * NSH]
        sp = np.concatenate([s, np.ones(NSH_PAD - NSH, dtype=np.float32)])
        sqdeg_row[c, 0] = sp.astype(ml_dtypes.bfloat16)

    # x transposed shards, hi/lo bf16
    xT_hi = np.zeros((NCORES, 128, NSH_PAD), dtype=ml_dtypes.bfloat16)
    xT_lo = np.zeros((NCORES, 128, NSH_PAD), dtype=ml_dtypes.bfloat16)
    for c in range(NCORES):
        xs = np.asarray(x[c * NSH : (c + 1) * NSH]).astype(np.float32).T
        hi, lo = _split_hi_lo(xs)
        xT_hi[c, :, :NSH] = hi
        xT_lo[c, :, :NSH] = lo

    W0h, W0l = _split_hi_lo(np.asarray(W0, dtype=np.float32))
    W1h, W1l = _split_hi_lo(np.asarray(W1, dtype=np.float32))
    Wlh, Wll = _split_hi_lo(np.asarray(Wl, dtype=np.float32))
    b0r = np.asarray(b0, dtype=np.float32).reshape(1, HID).astype(ml_dtypes.bfloat16)
    b1r = np.asarray(b1, dtype=np.float32).reshape(1, HID).astype(ml_dtypes.bfloat16)
    blb12 = np.tile(np.asarray(bl, dtype=np.float32).reshape(1, C_OUT), (128, TG))

    in_maps = []
    for c in range(NCORES):
        in_maps.append(
            {
                "xT_hi": xT_hi[c],
                "xT_lo": xT_lo[c],
                "gidx": gidx_all[c],
                "dsl": dsl_all[c],
                "dinv_col": dinv_col[c],
                "dinv_b": dinv_b[c],
                "sqdeg_row": sqdeg_row[c],
                "W0h": W0h, "W0l": W0l,
                "W1h": W1h, "W1l": W1l,
                "Wlh": Wlh, "Wll": Wll,
                "b0r": b0r, "b1r": b1r, "blb12": blb12,
            }
        )
    return in_maps, nblk_tk, tot_blocks, tot_slots


def _build_program(nblk_tk, tot_blocks, tot_slots):
    nc = bacc.Bacc(num_devices=NCORES, num_swdge_queues=NQ)
    xT_hi = nc.declare_dram_parameter("xT_hi", [128, NSH_PAD], BF16, isOutput=False)
    xT_lo = nc.declare_dram_parameter("xT_lo", [128, NSH_PAD], BF16, isOutput=False)
    gidx = nc.declare_dram_parameter("gidx", [128, tot_slots // 16], I16, isOutput=False)
    dsl = nc.declare_dram_parameter("dsl", [128, tot_blocks], BF16, isOutput=False)
    dinv_col_e = nc.declare_dram_parameter("dinv_col", [128, NT], F32, isOutput=False)
    dinv_b_e = nc.declare_dram_parameter("dinv_b", [128, NSH_PAD], BF16, isOutput=False)
    sqdeg_e = nc.declare_dram_parameter("sqdeg_row", [1, NSH_PAD], BF16, isOutput=False)
    W0h = nc.declare_dram_parameter("W0h", [128, HID], BF16, isOutput=False)
    W0l = nc.declare_dram_parameter("W0l", [128, HID], BF16, isOutput=False)
    W1h = nc.declare_dram_parameter("W1h", [128, HID], BF16, isOutput=False)
    W1l = nc.declare_dram_parameter("W1l", [128, HID], BF16, isOutput=False)
    Wlh = nc.declare_dram_parameter("Wlh", [128, C_OUT], BF16, isOutput=False)
    Wll = nc.declare_dram_parameter("Wll", [128, C_OUT], BF16, isOutput=False)
    b0r_e = nc.declare_dram_parameter("b0r", [1, HID], BF16, isOutput=False)
    b1r_e = nc.declare_dram_parameter("b1r", [1, HID], BF16, isOutput=False)
    blb12_e = nc.declare_dram_parameter("blb12", [128, TG * C_OUT], F32, isOutput=False)
    out_ext = nc.declare_dram_parameter("out", [NSH, C_OUT], F32, isOutput=True)

    t1_shard = nc.dram_tensor("t1_shard", [NSH, HID], BF16)
    t2_shard = nc.dram_tensor("t2_shard", [NSH, HID], BF16)
    T1_full = nc.dram_tensor("T1_full", [N, HID], BF16, addr_space="Shared")
    T2_full = nc.dram_tensor("T2_full", [N, HID], BF16, addr_space="Shared")

    # per-(tg,k) slot counts
    sgk = np.zeros((NTG, NCHUNK), dtype=np.int64)
    for g in range(NTG):
        tiles = range(g * TG, min((g + 1) * TG, NT))
        for k in range(NCHUNK):
            sgk[g][k] = P * sum(int(nblk_tk[t][k]) for t in tiles)
    max_gk_blocks = int(sgk.max()) // P

    qctr = [0]  # SWDGE queue round-robin

    from contextlib import ExitStack
    with tile.TileContext(nc) as tc, ExitStack() as es:
        cpool = es.enter_context(tc.tile_pool(name="const", bufs=1))
        xpool = es.enter_context(tc.tile_pool(name="xp", bufs=3))
        gpool = es.enter_context(tc.tile_pool(name="gp", bufs=4))
        ipool = es.enter_context(tc.tile_pool(name="ip", bufs=4))
        dpool = es.enter_context(tc.tile_pool(name="dp", bufs=2))
        spool = es.enter_context(tc.tile_pool(name="sp", bufs=3))
        zpool = es.enter_context(tc.tile_pool(name="zp", bufs=1))
        opool = es.enter_context(tc.tile_pool(name="op", bufs=2))
        apsum = es.enter_context(tc.tile_pool(name="apsum", bufs=2, space="PSUM"))
        wpsum = es.enter_context(tc.tile_pool(name="wpsum", bufs=2, space="PSUM"))

        # ---- constants ----
        iota_t = cpool.tile([P, P], BF16, tag="iota")
        nc.gpsimd.iota(iota_t[:], pattern=[[1, P]], base=0, channel_multiplier=0,
                       allow_small_or_imprecise_dtypes=True)
        w0h_t = cpool.tile([128, HID], BF16, tag="w0h")
        w0l_t = cpool.tile([128, HID], BF16, tag="w0l")
        w1h_t = cpool.tile([128, HID], BF16, tag="w1h")
        w1l_t = cpool.tile([128, HID], BF16, tag="w1l")
        wlh_t = cpool.tile([128, C_OUT], BF16, tag="wlh")
        wll_t = cpool.tile([128, C_OUT], BF16, tag="wll")
        b0_t = cpool.tile([1, HID], BF16, tag="b0")
        b1_t = cpool.tile([1, HID], BF16, tag="b1")
        blb_t = cpool.tile([128, TG * C_OUT], F32, tag="blb")
        dinv_col = cpool.tile([128, NT], F32, tag="dcol")
        dinv_b = cpool.tile([128, NSH_PAD], BF16, tag="dinvb")
        sqdeg_t = cpool.tile([1, NSH_PAD], BF16, tag="sqdeg")
        for tt, ext in [(w0h_t, W0h), (w0l_t, W0l), (w1h_t, W1h), (w1l_t, W1l),
                        (wlh_t, Wlh), (wll_t, Wll), (b0_t, b0r_e), (b1_t, b1r_e),
                        (blb_t, blb12_e), (dinv_col, dinv_col_e),
                        (dinv_b, dinv_b_e), (sqdeg_t, sqdeg_e)]:
            nc.sync.dma_start(out=tt[:], in_=ext[:, :])

        # ---- phase 1: T1 shard = dinv * (X @ W0) ----
        for t in range(NT):
            rows = min(P, NSH - t * P)
            xh = xpool.tile([128, P], BF16, tag="xh")
            xl = xpool.tile([128, P], BF16, tag="xl")
            nc.sync.dma_start(out=xh[:], in_=xT_hi[:, t * P : (t + 1) * P])
            nc.sync.dma_start(out=xl[:], in_=xT_lo[:, t * P : (t + 1) * P])
            ps = wpsum.tile([P, 512], F32, tag="wps", space="PSUM")
            nc.tensor.matmul(out=ps[:, :HID], lhsT=xh[:], rhs=w0h_t[:], start=True,
                             stop=False, skip_group_check=True)
            nc.tensor.matmul(out=ps[:, :HID], lhsT=xh[:], rhs=w0l_t[:], start=False,
                             stop=False, skip_group_check=True)
            nc.tensor.matmul(out=ps[:, :HID], lhsT=xl[:], rhs=w0h_t[:], start=False,
                             stop=True, skip_group_check=True)
            tb = opool.tile([P, HID], BF16, tag="tb")
            nc.vector.tensor_scalar(out=tb[:], in0=ps[:, :HID],
                                    scalar1=dinv_col[:, t : t + 1],
                                    scalar2=None, op0=mybir.AluOpType.mult)
            nc.sync.dma_start(out=t1_shard[t * P : t * P + rows, :], in_=tb[:rows, :])

        # ---- allgather T1 ----
        nc.gpsimd.collective_compute(
            "AllGather", mybir.AluOpType.bypass,
            replica_groups=[list(range(NCORES))],
            ins=[t1_shard[:].opt()], outs=[T1_full[:].opt()],
        )

        # ---- aggregation layers ----
        def agg_layer(T_full, layer):
            gcol = [0]
            bcol = [0]

            def emit_agg(g):
                tiles = list(range(g * TG, min((g + 1) * TG, NT)))
                ntl = len(tiles)
                gw = ntl * P            # group free width (nodes)
                goff = g * TG * P       # node offset of group start
                first_k = {}
                last_k = {}
                for t in tiles:
                    ks = [k for k in range(NCHUNK) if nblk_tk[t][k] > 0]
                    first_k[t], last_k[t] = ks[0], ks[-1]
                nbank = (ntl + 3) // 4
                banks = [apsum.tile([P, 512], F32, tag=f"agg{i}", space="PSUM",
                                    name=f"aggbank{i}")
                         for i in range(nbank)]

                def agg_ap(ti):
                    i = tiles.index(ti)
                    return banks[i // 4][:, (i % 4) * P : (i % 4 + 1) * P]

                # bias-fold: psum <- bias[feat] * sqrt(deg[node]); after the
                # dinv epilogue scale this is exactly +bias.
                bias_r = b0_t if layer == 1 else b1_t
                for i in range(nbank):
                    bw = min(512, gw - i * 512)
                    nc.tensor.matmul(
                        out=banks[i][:, :bw], lhsT=bias_r[:],
                        rhs=sqdeg_t[:, goff + i * 512 : goff + i * 512 + bw],
                        start=True, stop=False, skip_group_check=True)

                # dstslot slab for this tile group
                nb_tg = sum(int(nblk_tk[t][k]) for t in tiles for k in range(NCHUNK))
                dst_t = dpool.tile([128, nb_tg], BF16, tag="dsl")
                nc.sync.dma_start(out=dst_t[:], in_=dsl[:, bcol[0] : bcol[0] + nb_tg])

                dcol_off = 0

                def onehot_batch(c0, cnt):
                    s_t = spool.tile([P, OB, P], BF16, tag="onehot")
                    nc.vector.tensor_tensor(
                        out=s_t[:, :cnt, :],
                        in0=iota_t[:].unsqueeze(1).broadcast_to([P, cnt, P]),
                        in1=dst_t[:, c0 : c0 + cnt].unsqueeze(2)
                            .broadcast_to([P, cnt, P]),
                        op=mybir.AluOpType.is_equal,
                    )
                    return s_t

                for k in range(NCHUNK):
                    s_gk = int(sgk[g][k])
                    if s_gk == 0:
                        continue
                    kend = min((k + 1) * CH, N)
                    idxt = ipool.tile([128, max(int(sgk.max()) // 16, 16)], I16,
                                      tag="gidx")
                    nc.sync.dma_start(out=idxt[:, : s_gk // 16],
                                      in_=gidx[:, gcol[0] : gcol[0] + s_gk // 16])
                    gbuf = gpool.tile([P, max_gk_blocks, P], BF16, tag="gath")
                    nblk_gk = s_gk // P
                    # split into two half-gathers on different SWDGE queues
                    h = (nblk_gk + 1) // 2
                    for b0, b1 in ((0, h), (h, nblk_gk)):
                        if b1 <= b0:
                            continue
                        s_h = (b1 - b0) * P
                        nc.gpsimd.dma_gather(
                            gbuf[:, b0:b1, :], T_full[k * CH : kend, :],
                            idxt[:, b0 * 8 : b0 * 8 + s_h // 16], s_h, s_h, HID,
                            single_packet=False, queue_num=qctr[0] % NQ,
                        )
                        qctr[0] += 1
                    gcol[0] += s_gk // 16
                    boff = 0
                    batch = None
                    batch_c0 = -1
                    for t in tiles:
                        nb = int(nblk_tk[t][k])
                        for b in range(nb):
                            if batch is None or dcol_off - batch_c0 >= OB:
                                cnt = min(OB, nb_tg - dcol_off)
                                batch = onehot_batch(dcol_off, cnt)
                                batch_c0 = dcol_off
                            nc.tensor.matmul(
                                out=agg_ap(t), lhsT=gbuf[:, boff, :],
                                rhs=batch[:, dcol_off - batch_c0, :],
                                start=False,
                                stop=(k == last_k[t] and b == nb - 1),
                                skip_group_check=True,
                            )
                            boff += 1
                            dcol_off += 1
                bcol[0] += nb_tg
                return banks, tiles, ntl, gw, goff

            def emit_epilogue(ctx):
                banks, tiles, ntl, gw, goff = ctx
                nbank = (ntl + 3) // 4
                # ---- batched epilogue: u = psum*dinv; ELU; zb (bf16) ----
                u = zpool.tile([P, TG * P], F32, tag="u")
                for i in range(nbank):
                    bw = min(512, gw - i * 512)
                    nc.vector.tensor_tensor(
                        out=u[:, i * 512 : i * 512 + bw], in0=banks[i][:, :bw],
                        in1=dinv_b[:, goff + i * 512 : goff + i * 512 + bw],
                        op=mybir.AluOpType.mult)
                mn = zpool.tile([P, TG * P], F32, tag="mn")
                nc.vector.tensor_scalar(out=mn[:, :gw], in0=u[:, :gw], scalar1=0.0,
                                        scalar2=None, op0=mybir.AluOpType.min)
                ex = zpool.tile([P, TG * P], F32, tag="ex")
                nc.scalar.activation(ex[:, :gw], mn[:, :gw],
                                     mybir.ActivationFunctionType.Exp)
                px = zpool.tile([P, TG * P], F32, tag="px")
                nc.vector.tensor_scalar(out=px[:, :gw], in0=u[:, :gw], scalar1=0.0,
                                        scalar2=None, op0=mybir.AluOpType.max)
                nc.vector.tensor_tensor(out=mn[:, :gw], in0=px[:, :gw],
                                        in1=ex[:, :gw], op=mybir.AluOpType.add)
                zb = zpool.tile([P, TG * P], BF16, tag="zb")
                nc.vector.tensor_scalar(out=zb[:, :gw], in0=mn[:, :gw], scalar1=-1.0,
                                        scalar2=None, op0=mybir.AluOpType.add)

                if layer == 1:
                    for i, t in enumerate(tiles):
                        rows = min(P, NSH - t * P)
                        ps2 = wpsum.tile([P, 512], F32, tag="wps", space="PSUM")
                        nc.tensor.matmul(out=ps2[:, :HID],
                                         lhsT=zb[:, i * P : (i + 1) * P],
                                         rhs=w1h_t[:], start=True, stop=False,
                                         skip_group_check=True)
                        nc.tensor.matmul(out=ps2[:, :HID],
                                         lhsT=zb[:, i * P : (i + 1) * P],
                                         rhs=w1l_t[:], start=False, stop=True,
                                         skip_group_check=True)
                        t2b = opool.tile([P, HID], BF16, tag="tb")
                        nc.vector.tensor_scalar(out=t2b[:], in0=ps2[:, :HID],
                                                scalar1=dinv_col[:, t : t + 1],
                                                scalar2=None,
                                                op0=mybir.AluOpType.mult)
                        nc.sync.dma_start(out=t2_shard[t * P : t * P + rows, :],
                                          in_=t2b[:rows, :])
                else:
                    cls = wpsum.tile([P, 512], F32, tag="wps", space="PSUM")
                    for i, t in enumerate(tiles):
                        nc.tensor.matmul(out=cls[:, i * C_OUT : (i + 1) * C_OUT],
                                         lhsT=zb[:, i * P : (i + 1) * P],
                                         rhs=wlh_t[:], start=True, stop=False,
                                         skip_group_check=True)
                        nc.tensor.matmul(out=cls[:, i * C_OUT : (i + 1) * C_OUT],
                                         lhsT=zb[:, i * P : (i + 1) * P],
                                         rhs=wll_t[:], start=False, stop=True,
                                         skip_group_check=True)
                    cw = ntl * C_OUT
                    lg = opool.tile([P, TG * C_OUT], F32, tag="lg")
                    nc.vector.tensor_tensor(out=lg[:, :cw], in0=cls[:, :cw],
                                            in1=blb_t[:, :cw],
                                            op=mybir.AluOpType.add)
                    ex2 = opool.tile([P, TG * C_OUT], F32, tag="ex2")
                    nc.scalar.activation(ex2[:, :cw], lg[:, :cw],
                                         mybir.ActivationFunctionType.Exp)
                    sm = opool.tile([P, TG], F32, tag="sm")
                    nc.vector.tensor_reduce(
                        out=sm[:, :ntl],
                        in_=ex2[:, :cw].rearrange("p (t c) -> p t c", c=C_OUT),
                        axis=mybir.AxisListType.X, op=mybir.AluOpType.add)
                    ln = opool.tile([P, TG], F32, tag="ln")
                    nc.scalar.activation(ln[:, :ntl], sm[:, :ntl],
                                         mybir.ActivationFunctionType.Ln)
                    res = opool.tile([P, TG * C_OUT], F32, tag="res")
                    nc.vector.tensor_tensor(
                        out=res[:, :cw].rearrange("p (t c) -> p t c", c=C_OUT),
                        in0=lg[:, :cw].rearrange("p (t c) -> p t c", c=C_OUT),
                        in1=ln[:, :ntl].unsqueeze(2).broadcast_to([P, ntl, C_OUT]),
                        op=mybir.AluOpType.subtract)
                    for i, t in enumerate(tiles):
                        rows = min(P, NSH - t * P)
                        nc.sync.dma_start(
                            out=out_ext[t * P : t * P + rows, :],
                            in_=res[:rows, i * C_OUT : (i + 1) * C_OUT])

            # software pipeline: epilogue of group g runs one group behind
            # the aggregation stream so Vector never head-of-line blocks.
            pending = []
            for g in range(NTG):
                pending.append(emit_agg(g))
                if len(pending) == 2:
                    emit_epilogue(pending.pop(0))
            while pending:
                emit_epilogue(pending.pop(0))

        agg_layer(T1_full, 1)
        nc.gpsimd.collective_compute(
            "AllGather", mybir.AluOpType.bypass,
            replica_groups=[list(range(NCORES))],
            ins=[t2_shard[:].opt()], outs=[T2_full[:].opt()],
        )
        agg_layer(T2_full, 2)

    nc.finalize()
    return nc


_CACHE = {}


def kernel(**inputs):
    in_maps, nblk_tk, tot_blocks, tot_slots = _prep_host(
        inputs["x"], inputs["edge_index"], inputs["W0"], inputs["b0"],
        inputs["W1"], inputs["b1"], inputs["Wl"], inputs["bl"])
    key = (tot_blocks, tot_slots, nblk_tk.tobytes())
    if key not in _CACHE:
        _CACHE[key] = _build_program(nblk_tk, tot_blocks, tot_slots)
    nc = _CACHE[key]
    trace = bool(int(__import__("os").environ.get("KERNEL_TRACE", "0")))
    res = run_bass_kernel_spmd(nc, in_maps, list(range(NCORES)), trace=trace)
    kernel.last_results = res
    out = np.concatenate([res.results[c]["out"] for c in range(NCORES)], axis=0)
    return out.astype(np.float32)


# revision 10
# speedup vs baseline: 1.6139x; 1.0043x over previous
"""2-layer GCN + classifier on 8 Trainium2 NeuronCores.

Strategy (graph/data parallel):
- Nodes sharded 8 ways by contiguous range (12500/core). Edges partitioned by
  dst shard on host, grouped by (dst-tile of 128, src-chunk of 32768), padded
  to 128-edge blocks. Self-loops appended as ordinary edges.
- Per GCN layer: each core builds its shard of the gather table
  T = dinv * (Z @ W) (bf16), AllGather -> full table in every core's HBM.
  Aggregation: dma_gather rows by src (int16 chunk-local idx) spread over the
  4 SWDGE queues (overlaps descriptor generation with drains), one-hot
  dst-slot matrices built in wide batches via broadcast-AP is_equal,
  segment-sum via PSUM-accumulated bf16 matmuls. Bias (pre-scaled by
  sqrt(deg)) seeds the PSUM banks via a rank-1 matmul so the epilogue is just
  dinv-scale + ELU, batched per 12-tile group.
- Classifier + log_softmax computed per tile group; host concatenates the 8
  output shards.
"""
import sys

sys.path.insert(0, "/opt/trn_rl_repo")

import numpy as np
import ml_dtypes

import concourse.bacc as bacc
import concourse.tile as tile
from concourse import mybir
from concourse.bass_utils import run_bass_kernel_spmd

# ---------------- problem constants (hardcoded per task statement) ----------
N = 100000
E = 1600000
F_IN = 128
HID = 128
C_OUT = 40
NCORES = 8
NSH = N // NCORES          # 12500 nodes per core
P = 128
NT = (NSH + P - 1) // P    # 98 dst tiles per core (last has 84 rows)
NSH_PAD = NT * P           # 12544
CH = 32768                 # gather chunk rows (int16 idx limit)
NCHUNK = (N + CH - 1) // CH  # 4
TG = 12                    # dst tiles per tile-group (PSUM: 3 banks x 2 bufs)
NTG = (NT + TG - 1) // TG  # 9 tile groups
OB = 16                    # one-hot build batch (blocks per DVE op)
NQ = 4                     # SWDGE queues for gathers

F32 = mybir.dt.float32
BF16 = mybir.dt.bfloat16
I16 = mybir.dt.int16


def _split_hi_lo(w):
    hi = w.astype(ml_dtypes.bfloat16)
    lo = (w - hi.astype(np.float32)).astype(ml_dtypes.bfloat16)
    return hi, lo


def _prep_host(x, edge_index, W0, b0, W1, b1, Wl, bl):
    """Shard + reorder edges; build all per-core device input arrays."""
    src = np.asarray(edge_index[0]).astype(np.int64)
    dst = np.asarray(edge_index[1]).astype(np.int64)
    loop = np.arange(N, dtype=np.int64)
    src2 = np.concatenate([src, loop])
    dst2 = np.concatenate([dst, loop])
    deg = np.bincount(dst2, minlength=N).astype(np.float32)  # = indeg + 1

    # ---- per-core edge grouping by (dst_tile, src_chunk) ----
    counts = np.zeros((NCORES, NT, NCHUNK), dtype=np.int64)
    grouped = []  # per core: (sorted_src, dst_slot, cnt)
    core_of = dst2 // NSH
    for c in range(NCORES):
        sel = core_of == c
        es = src2[sel]
        ed = dst2[sel] - c * NSH
        tile_id = ed // P
        chunk_id = es // CH
        key = tile_id * NCHUNK + chunk_id
        order = np.argsort(key, kind="stable")
        es, ed, key = es[order], ed[order], key[order]
        cnt = np.bincount(key, minlength=NT * NCHUNK).reshape(NT, NCHUNK)
        counts[c] = cnt
        grouped.append((es, ed % P, cnt))

    nblk_tk = np.ceil(counts.max(axis=0) / P).astype(np.int64)  # [NT, NCHUNK]

    # flatten order: for tg: for k: for t in tg: blocks
    tot_blocks = 0
    for g in range(NTG):
        tiles = range(g * TG, min((g + 1) * TG, NT))
        for k in range(NCHUNK):
            for t in tiles:
                tot_blocks += int(nblk_tk[t][k])
    tot_slots = tot_blocks * P

    gidx_all = np.zeros((NCORES, 128, tot_slots // 16), dtype=np.int16)
    dsl_all = np.full((NCORES, 128, tot_blocks), -1.0, dtype=ml_dtypes.bfloat16)

    for c in range(NCORES):
        es, slots, cnt = grouped[c]
        starts = np.zeros(NT * NCHUNK + 1, dtype=np.int64)
        np.cumsum(cnt.reshape(-1), out=starts[1:])
        gcol = 0  # gidx column offset (16-wide units)
        bcol = 0  # dstslot block column offset
        for g in range(NTG):
            tiles = range(g * TG, min((g + 1) * TG, NT))
            for k in range(NCHUNK):
                for t in tiles:
                    nb = int(nblk_tk[t][k])
                    if nb == 0:
                        continue
                    a, b = starts[t * NCHUNK + k], starts[t * NCHUNK + k + 1]
                    n = b - a
                    pad = nb * P - n
                    sl = np.concatenate(
                        [es[a:b] - k * CH, np.zeros(pad, dtype=np.int64)]
                    ).astype(np.int16)
                    ds = np.concatenate(
                        [slots[a:b], np.full(pad, -1, dtype=np.int64)]
                    ).astype(ml_dtypes.bfloat16)
                    # wrap idx: slot i -> [i%16, i//16]
                    gidx_all[c, :16, gcol : gcol + nb * 8] = sl.reshape(-1, 16).T
                    dsl_all[c, :, bcol : bcol + nb] = ds.reshape(nb, P).T
                    gcol += nb * 8
                    bcol += nb
        gidx_all[c] = np.tile(gidx_all[c, :16], (8, 1))

    # degree-derived tables (host-precomputed)
    dinv = np.where(deg > 0, 1.0 / np.sqrt(deg), 0.0).astype(np.float32)
    sqdeg = np.sqrt(deg).astype(np.float32)
    dinv_col = np.ones((NCORES, 128, NT), dtype=np.float32)
    dinv_b = np.ones((NCORES, 128, NSH_PAD), dtype=ml_dtypes.bfloat16)
    sqdeg_row = np.ones((NCORES, 1, NSH_PAD), dtype=ml_dtypes.bfloat16)
    for c in range(NCORES):
        d = dinv[c * NSH : (c + 1) * NSH]
        dp = np.concatenate([d, np.ones(NSH_PAD - NSH, dtype=np.float32)])
        dinv_col[c] = dp.reshape(NT, P).T
        dinv_b[c] = np.broadcast_to(dp.astype(ml_dtypes.bfloat16), (128, NSH_PAD))
        s = sqdeg[c * NSH : (c + 1) * NSH]
        sp = np.concatenate([s, np.ones(NSH_PAD - NSH, dtype=np.float32)])
        sqdeg_row[c, 0] = sp.astype(ml_dtypes.bfloat16)

    # x transposed shards, hi/lo bf16
    xT_hi = np.zeros((NCORES, 128, NSH_PAD), dtype=ml_dtypes.bfloat16)
    xT_lo = np.zeros((NCORES, 128, NSH_PAD), dtype=ml_dtypes.bfloat16)
    for c in range(NCORES):
        xs = np.asarray(x[c * NSH : (c + 1) * NSH]).astype(np.float32).T
        hi, lo = _split_hi_lo(xs)
        xT_hi[c, :, :NSH] = hi
        xT_lo[c, :, :NSH] = lo

    W0h, W0l = _split_hi_lo(np.asarray(W0, dtype=np.float32))
    W1h, W1l = _split_hi_lo(np.asarray(W1, dtype=np.float32))
    Wlh, Wll = _split_hi_lo(np.asarray(Wl, dtype=np.float32))
    b0r = np.asarray(b0, dtype=np.float32).reshape(1, HID).astype(ml_dtypes.bfloat16)
    b1r = np.asarray(b1, dtype=np.float32).reshape(1, HID).astype(ml_dtypes.bfloat16)
    blb12 = np.tile(np.asarray(bl, dtype=np.float32).reshape(1, C_OUT), (128, TG))

    in_maps = []
    for c in range(NCORES):
        in_maps.append(
            {
                "xT_hi": xT_hi[c],
                "xT_lo": xT_lo[c],
                "gidx": gidx_all[c],
                "dsl": dsl_all[c],
                "dinv_col": dinv_col[c],
                "dinv_b": dinv_b[c],
                "sqdeg_row": sqdeg_row[c],
                "W0h": W0h, "W0l": W0l,
                "W1h": W1h, "W1l": W1l,
                "Wlh": Wlh, "Wll": Wll,
                "b0r": b0r, "b1r": b1r, "blb12": blb12,
            }
        )
    return in_maps, nblk_tk, tot_blocks, tot_slots


def _build_program(nblk_tk, tot_blocks, tot_slots):
    nc = bacc.Bacc(num_devices=NCORES, num_swdge_queues=NQ)
    xT_hi = nc.declare_dram_parameter("xT_hi", [128, NSH_PAD], BF16, isOutput=False)
    xT_lo = nc.declare_dram_parameter("xT_lo", [128, NSH_PAD], BF16, isOutput=False)
    gidx = nc.declare_dram_parameter("gidx", [128, tot_slots // 16], I16, isOutput=False)
    dsl = nc.declare_dram_parameter("dsl", [128, tot_blocks], BF16, isOutput=False)
    dinv_col_e = nc.declare_dram_parameter("dinv_col", [128, NT], F32, isOutput=False)
    dinv_b_e = nc.declare_dram_parameter("dinv_b", [128, NSH_PAD], BF16, isOutput=False)
    sqdeg_e = nc.declare_dram_parameter("sqdeg_row", [1, NSH_PAD], BF16, isOutput=False)
    W0h = nc.declare_dram_parameter("W0h", [128, HID], BF16, isOutput=False)
    W0l = nc.declare_dram_parameter("W0l", [128, HID], BF16, isOutput=False)
    W1h = nc.declare_dram_parameter("W1h", [128, HID], BF16, isOutput=False)
    W1l = nc.declare_dram_parameter("W1l", [128, HID], BF16, isOutput=False)
    Wlh = nc.declare_dram_parameter("Wlh", [128, C_OUT], BF16, isOutput=False)
    Wll = nc.declare_dram_parameter("Wll", [128, C_OUT], BF16, isOutput=False)
    b0r_e = nc.declare_dram_parameter("b0r", [1, HID], BF16, isOutput=False)
    b1r_e = nc.declare_dram_parameter("b1r", [1, HID], BF16, isOutput=False)
    blb12_e = nc.declare_dram_parameter("blb12", [128, TG * C_OUT], F32, isOutput=False)
    out_ext = nc.declare_dram_parameter("out", [NSH, C_OUT], F32, isOutput=True)

    t1_shard = nc.dram_tensor("t1_shard", [NSH, HID], BF16)
    t2_shard = nc.dram_tensor("t2_shard", [NSH, HID], BF16)
    T1_full = nc.dram_tensor("T1_full", [N, HID], BF16, addr_space="Shared")
    T2_full = nc.dram_tensor("T2_full", [N, HID], BF16, addr_space="Shared")

    # per-(tg,k) slot counts
    sgk = np.zeros((NTG, NCHUNK), dtype=np.int64)
    for g in range(NTG):
        tiles = range(g * TG, min((g + 1) * TG, NT))
        for k in range(NCHUNK):
            sgk[g][k] = P * sum(int(nblk_tk[t][k]) for t in tiles)
    max_gk_blocks = int(sgk.max()) // P

    qctr = [0]  # SWDGE queue round-robin

    from contextlib import ExitStack
    with tile.TileContext(nc) as tc, ExitStack() as es:
        cpool = es.enter_context(tc.tile_pool(name="const", bufs=1))
        xpool = es.enter_context(tc.tile_pool(name="xp", bufs=3))
        gpool = es.enter_context(tc.tile_pool(name="gp", bufs=4))
        ipool = es.enter_context(tc.tile_pool(name="ip", bufs=4))
        dpool = es.enter_context(tc.tile_pool(name="dp", bufs=2))
        spool = es.enter_context(tc.tile_pool(name="sp", bufs=5))
        zpool = es.enter_context(tc.tile_pool(name="zp", bufs=1))
        opool = es.enter_context(tc.tile_pool(name="op", bufs=2))
        apsum = es.enter_context(tc.tile_pool(name="apsum", bufs=2, space="PSUM"))
        wpsum = es.enter_context(tc.tile_pool(name="wpsum", bufs=2, space="PSUM"))

        # ---- constants ----
        iota_t = cpool.tile([P, P], BF16, tag="iota")
        nc.gpsimd.iota(iota_t[:], pattern=[[1, P]], base=0, channel_multiplier=0,
                       allow_small_or_imprecise_dtypes=True)
        w0h_t = cpool.tile([128, HID], BF16, tag="w0h")
        w0l_t = cpool.tile([128, HID], BF16, tag="w0l")
        w1h_t = cpool.tile([128, HID], BF16, tag="w1h")
        w1l_t = cpool.tile([128, HID], BF16, tag="w1l")
        wlh_t = cpool.tile([128, C_OUT], BF16, tag="wlh")
        wll_t = cpool.tile([128, C_OUT], BF16, tag="wll")
        b0_t = cpool.tile([1, HID], BF16, tag="b0")
        b1_t = cpool.tile([1, HID], BF16, tag="b1")
        blb_t = cpool.tile([128, TG * C_OUT], F32, tag="blb")
        dinv_col = cpool.tile([128, NT], F32, tag="dcol")
        dinv_b = cpool.tile([128, NSH_PAD], BF16, tag="dinvb")
        sqdeg_t = cpool.tile([1, NSH_PAD], BF16, tag="sqdeg")
        for tt, ext in [(w0h_t, W0h), (w0l_t, W0l), (w1h_t, W1h), (w1l_t, W1l),
                        (wlh_t, Wlh), (wll_t, Wll), (b0_t, b0r_e), (b1_t, b1r_e),
                        (blb_t, blb12_e), (dinv_col, dinv_col_e),
                        (dinv_b, dinv_b_e), (sqdeg_t, sqdeg_e)]:
            nc.sync.dma_start(out=tt[:], in_=ext[:, :])

        # ---- phase 1: T1 shard = dinv * (X @ W0) ----
        for t in range(NT):
            rows = min(P, NSH - t * P)
            xh = xpool.tile([128, P], BF16, tag="xh")
            xl = xpool.tile([128, P], BF16, tag="xl")
            nc.sync.dma_start(out=xh[:], in_=xT_hi[:, t * P : (t + 1) * P])
            nc.sync.dma_start(out=xl[:], in_=xT_lo[:, t * P : (t + 1) * P])
            ps = wpsum.tile([P, 512], F32, tag="wps", space="PSUM")
            nc.tensor.matmul(out=ps[:, :HID], lhsT=xh[:], rhs=w0h_t[:], start=True,
                             stop=False, skip_group_check=True)
            nc.tensor.matmul(out=ps[:, :HID], lhsT=xh[:], rhs=w0l_t[:], start=False,
                             stop=False, skip_group_check=True)
            nc.tensor.matmul(out=ps[:, :HID], lhsT=xl[:], rhs=w0h_t[:], start=False,
                             stop=True, skip_group_check=True)
            tb = opool.tile([P, HID], BF16, tag="tb")
            nc.vector.tensor_scalar(out=tb[:], in0=ps[:, :HID],
                                    scalar1=dinv_col[:, t : t + 1],
                                    scalar2=None, op0=mybir.AluOpType.mult)
            nc.sync.dma_start(out=t1_shard[t * P : t * P + rows, :], in_=tb[:rows, :])

        # ---- allgather T1 ----
        nc.gpsimd.collective_compute(
            "AllGather", mybir.AluOpType.bypass,
            replica_groups=[list(range(NCORES))],
            ins=[t1_shard[:].opt()], outs=[T1_full[:].opt()],
        )

        # ---- aggregation layers ----
        def agg_layer(T_full, layer):
            gcol = [0]
            bcol = [0]

            def emit_agg(g):
                tiles = list(range(g * TG, min((g + 1) * TG, NT)))
                ntl = len(tiles)
                gw = ntl * P            # group free width (nodes)
                goff = g * TG * P       # node offset of group start
                first_k = {}
                last_k = {}
                for t in tiles:
                    ks = [k for k in range(NCHUNK) if nblk_tk[t][k] > 0]
                    first_k[t], last_k[t] = ks[0], ks[-1]
                nbank = (ntl + 3) // 4
                banks = [apsum.tile([P, 512], F32, tag=f"agg{i}", space="PSUM",
                                    name=f"aggbank{i}")
                         for i in range(nbank)]

                def agg_ap(ti):
                    i = tiles.index(ti)
                    return banks[i // 4][:, (i % 4) * P : (i % 4 + 1) * P]

                # bias-fold: psum <- bias[feat] * sqrt(deg[node]); after the
                # dinv epilogue scale this is exactly +bias.
                bias_r = b0_t if layer == 1 else b1_t
                for i in range(nbank):
                    bw = min(512, gw - i * 512)
                    nc.tensor.matmul(
                        out=banks[i][:, :bw], lhsT=bias_r[:],
                        rhs=sqdeg_t[:, goff + i * 512 : goff + i * 512 + bw],
                        start=True, stop=False, skip_group_check=True)

                # dstslot slab for this tile group
                nb_tg = sum(int(nblk_tk[t][k]) for t in tiles for k in range(NCHUNK))
                dst_t = dpool.tile([128, nb_tg], BF16, tag="dsl")
                nc.sync.dma_start(out=dst_t[:], in_=dsl[:, bcol[0] : bcol[0] + nb_tg])

                dcol_off = 0

                def onehot_batch(c0, cnt):
                    s_t = spool.tile([P, OB, P], BF16, tag="onehot")
                    nc.vector.tensor_tensor(
                        out=s_t[:, :cnt, :],
                        in0=iota_t[:].unsqueeze(1).broadcast_to([P, cnt, P]),
                        in1=dst_t[:, c0 : c0 + cnt].unsqueeze(2)
                            .broadcast_to([P, cnt, P]),
                        op=mybir.AluOpType.is_equal,
                    )
                    return s_t

                for k in range(NCHUNK):
                    s_gk = int(sgk[g][k])
                    if s_gk == 0:
                        continue
                    kend = min((k + 1) * CH, N)
                    idxt = ipool.tile([128, max(int(sgk.max()) // 16, 16)], I16,
                                      tag="gidx")
                    nc.sync.dma_start(out=idxt[:, : s_gk // 16],
                                      in_=gidx[:, gcol[0] : gcol[0] + s_gk // 16])
                    gbuf = gpool.tile([P, max_gk_blocks, P], BF16, tag="gath")
                    nblk_gk = s_gk // P
                    # split into quarter-gathers on different SWDGE queues
                    qs = [nblk_gk * i // 4 for i in range(5)]
                    for b0, b1 in zip(qs[:-1], qs[1:]):
                        if b1 <= b0:
                            continue
                        s_h = (b1 - b0) * P
                        nc.gpsimd.dma_gather(
                            gbuf[:, b0:b1, :], T_full[k * CH : kend, :],
                            idxt[:, b0 * 8 : b0 * 8 + s_h // 16], s_h, s_h, HID,
                            single_packet=False, queue_num=qctr[0] % NQ,
                        )
                        qctr[0] += 1
                    gcol[0] += s_gk // 16
                    boff = 0
                    batch = None
                    batch_c0 = -1
                    for t in tiles:
                        nb = int(nblk_tk[t][k])
                        for b in range(nb):
                            if batch is None or dcol_off - batch_c0 >= OB:
                                cnt = min(OB, nb_tg - dcol_off)
                                batch = onehot_batch(dcol_off, cnt)
                                batch_c0 = dcol_off
                            nc.tensor.matmul(
                                out=agg_ap(t), lhsT=gbuf[:, boff, :],
                                rhs=batch[:, dcol_off - batch_c0, :],
                                start=False,
                                stop=(k == last_k[t] and b == nb - 1),
                                skip_group_check=True,
                            )
                            boff += 1
                            dcol_off += 1
                bcol[0] += nb_tg
                return banks, tiles, ntl, gw, goff

            def emit_epilogue(ctx):
                banks, tiles, ntl, gw, goff = ctx
                nbank = (ntl + 3) // 4
                # ---- batched epilogue: u = psum*dinv; ELU; zb (bf16) ----
                u = zpool.tile([P, TG * P], F32, tag="u")
                for i in range(nbank):
                    bw = min(512, gw - i * 512)
                    nc.vector.tensor_tensor(
                        out=u[:, i * 512 : i * 512 + bw], in0=banks[i][:, :bw],
                        in1=dinv_b[:, goff + i * 512 : goff + i * 512 + bw],
                        op=mybir.AluOpType.mult)
                mn = zpool.tile([P, TG * P], F32, tag="mn")
                nc.vector.tensor_scalar(out=mn[:, :gw], in0=u[:, :gw], scalar1=0.0,
                                        scalar2=None, op0=mybir.AluOpType.min)
                ex = zpool.tile([P, TG * P], F32, tag="ex")
                nc.scalar.activation(ex[:, :gw], mn[:, :gw],
                                     mybir.ActivationFunctionType.Exp)
                px = zpool.tile([P, TG * P], F32, tag="px")
                nc.vector.tensor_scalar(out=px[:, :gw], in0=u[:, :gw], scalar1=0.0,
                                        scalar2=None, op0=mybir.AluOpType.max)
                nc.vector.tensor_tensor(out=mn[:, :gw], in0=px[:, :gw],
                                        in1=ex[:, :gw], op=mybir.AluOpType.add)
                zb = zpool.tile([P, TG * P], BF16, tag="zb")
                nc.vector.tensor_scalar(out=zb[:, :gw], in0=mn[:, :gw], scalar1=-1.0,
                                        scalar2=None, op0=mybir.AluOpType.add)

                if layer == 1:
                    for i, t in enumerate(tiles):
                        rows = min(P, NSH - t * P)
                        ps2 = wpsum.tile([P, 512], F32, tag="wps", space="PSUM")
                        nc.tensor.matmul(out=ps2[:, :HID],
                                         lhsT=zb[:, i * P : (i + 1) * P],
                                         rhs=w1h_t[:], start=True, stop=False,
                                         skip_group_check=True)
                        nc.tensor.matmul(out=ps2[:, :HID],
                                         lhsT=zb[:, i * P : (i + 1) * P],
                                         rhs=w1l_t[:], start=False, stop=True,
                                         skip_group_check=True)
                        t2b = opool.tile([P, HID], BF16, tag="tb")
                        nc.vector.tensor_scalar(out=t2b[:], in0=ps2[:, :HID],
                                                scalar1=dinv_col[:, t : t + 1],
                                                scalar2=None,
                                                op0=mybir.AluOpType.mult)
                        nc.sync.dma_start(out=t2_shard[t * P : t * P + rows, :],
                                          in_=t2b[:rows, :])
                else:
                    cls = wpsum.tile([P, 512], F32, tag="wps", space="PSUM")
                    for i, t in enumerate(tiles):
                        nc.tensor.matmul(out=cls[:, i * C_OUT : (i + 1) * C_OUT],
                                         lhsT=zb[:, i * P : (i + 1) * P],
                                         rhs=wlh_t[:], start=True, stop=False,
                                         skip_group_check=True)
                        nc.tensor.matmul(out=cls[:, i * C_OUT : (i + 1) * C_OUT],
                                         lhsT=zb[:, i * P : (i + 1) * P],
                                         rhs=wll_t[:], start=False, stop=True,
                                         skip_group_check=True)
                    cw = ntl * C_OUT
                    lg = opool.tile([P, TG * C_OUT], F32, tag="lg")
                    nc.vector.tensor_tensor(out=lg[:, :cw], in0=cls[:, :cw],
                                            in1=blb_t[:, :cw],
                                            op=mybir.AluOpType.add)
                    ex2 = opool.tile([P, TG * C_OUT], F32, tag="ex2")
                    nc.scalar.activation(ex2[:, :cw], lg[:, :cw],
                                         mybir.ActivationFunctionType.Exp)
                    sm = opool.tile([P, TG], F32, tag="sm")
                    nc.vector.tensor_reduce(
                        out=sm[:, :ntl],
                        in_=ex2[:, :cw].rearrange("p (t c) -> p t c", c=C_OUT),
                        axis=mybir.AxisListType.X, op=mybir.AluOpType.add)
                    ln = opool.tile([P, TG], F32, tag="ln")
                    nc.scalar.activation(ln[:, :ntl], sm[:, :ntl],
                                         mybir.ActivationFunctionType.Ln)
                    res = opool.tile([P, TG * C_OUT], F32, tag="res")
                    nc.vector.tensor_tensor(
                        out=res[:, :cw].rearrange("p (t c) -> p t c", c=C_OUT),
                        in0=lg[:, :cw].rearrange("p (t c) -> p t c", c=C_OUT),
                        in1=ln[:, :ntl].unsqueeze(2).broadcast_to([P, ntl, C_OUT]),
                        op=mybir.AluOpType.subtract)
                    for i, t in enumerate(tiles):
                        rows = min(P, NSH - t * P)
                        nc.sync.dma_start(
                            out=out_ext[t * P : t * P + rows, :],
                            in_=res[:rows, i * C_OUT : (i + 1) * C_OUT])

            # software pipeline: epilogue of group g runs one group behind
            # the aggregation stream so Vector never head-of-line blocks.
            pending = []
            for g in range(NTG):
                pending.append(emit_agg(g))
                if len(pending) == 2:
                    emit_epilogue(pending.pop(0))
            while pending:
                emit_epilogue(pending.pop(0))

        agg_layer(T1_full, 1)
        nc.gpsimd.collective_compute(
            "AllGather", mybir.AluOpType.bypass,
            replica_groups=[list(range(NCORES))],
            ins=[t2_shard[:].opt()], outs=[T2_full[:].opt()],
        )
        agg_layer(T2_full, 2)

    nc.finalize()
    return nc


_CACHE = {}


def kernel(**inputs):
    in_maps, nblk_tk, tot_blocks, tot_slots = _prep_host(
        inputs["x"], inputs["edge_index"], inputs["W0"], inputs["b0"],
        inputs["W1"], inputs["b1"], inputs["Wl"], inputs["bl"])
    key = (tot_blocks, tot_slots, nblk_tk.tobytes())
    if key not in _CACHE:
        _CACHE[key] = _build_program(nblk_tk, tot_blocks, tot_slots)
    nc = _CACHE[key]
    trace = bool(int(__import__("os").environ.get("KERNEL_TRACE", "0")))
    res = run_bass_kernel_spmd(nc, in_maps, list(range(NCORES)), trace=trace)
    kernel.last_results = res
    out = np.concatenate([res.results[c]["out"] for c in range(NCORES)], axis=0)
    return out.astype(np.float32)
